# revision 1
# baseline (speedup 1.0000x reference)
"""EngramModule kernel for Trainium2 (8 NeuronCores, SPMD data-parallel).

Math (per token t, feature dim H=2048):
  idx[t, h]   = hash of n-gram ending at t (8 heads; computed on host, int64)
  memory[t]   = concat_h tables[h, idx[t, h]]                       (gather)
  key_raw     = memory @ Wk.T ;  value_raw = memory @ Wv.T          (GEMM)
  rsq_k       = rsqrt(mean(key_raw^2) + eps)   (rmsnorm scale, per token)
  rsq_v       = rsqrt(mean(value_raw^2) + eps)
  gate        = sigmoid(dot(hidden*key_norm_w, key_raw) * rsq_k / sqrt(H) - 4)
  g[t]        = gate * rsq_v * value_raw[t]          (= gated, value_norm_w folded)
  out[t]      = g[t]*(1+w2) + w1*g[t-1] + w0*g[t-2]  (causal depthwise conv k=3)

Device layout: tokens on partitions [128/tile], features on free axis.
Tables are cast bf16 on host (accuracy-neutral: the GEMM operand is bf16
anyway); memory is transposed for the PE contraction via bf16 PE transposes.
GEMMs in bf16 (weights resident in SBUF, k-slab layout), accumulate fp32 PSUM.
Conv shifts across partitions via PE shift-matrix matmuls; the 2-token
cross-core boundary is recomputed as a tiny 2-row "halo" tile (cores whose
shard starts at a sequence start get zero halo via idx=0 -> zero rows).
"""

import sys

import numpy as np

try:
    import concourse.bass as bass  # noqa: F401
except ImportError:
    sys.path.insert(0, "/opt/trn_rl_repo")

import concourse.bacc as bacc
import concourse.bass as bass
import concourse.tile as tile
from concourse import mybir
from concourse.bass_utils import run_bass_kernel_spmd

F32 = mybir.dt.float32
F32R = mybir.dt.float32r
BF16 = mybir.dt.bfloat16
I32 = mybir.dt.int32

P = 128
H = 2048          # hidden / memory dim
HEADS = 8
HEAD_DIM = 256
VOCAB = 65536
MODULUS = VOCAB - 1
EPS = 1e-6
GATE_BIAS = -4.0
N_CORES = 8
B, S = 4, 4096
TOK_PER_CORE = (B * S) // N_CORES   # 2048
NT = TOK_PER_CORE // P              # 16 token tiles per core
KT = H // P                         # 16 contraction slabs


# ---------------------------------------------------------------- host hashing
def _hash_ids_np(ids, mult, off, n):
    """Exact replica of the reference _hash_ids in numpy (wrapping int64)."""
    Bb, Ss = ids.shape
    nh = mult.shape[0]
    ids_u = ids.astype(np.uint64)
    mult_u = mult.astype(np.uint64)
    off_u = off.astype(np.uint64)
    mix = np.zeros((Bb, Ss, nh), dtype=np.uint64)
    for p in range(n):
        shift = n - 1 - p
        tok = np.zeros_like(ids_u)
        if shift > 0:
            tok[:, shift:] = ids_u[:, : Ss - shift]
        else:
            tok = ids_u
        mix ^= tok[:, :, None] * mult_u[None, None, :, p]
    h = (mix + off_u[None, None, :]).view(np.int64)
    hmod = np.remainder(h, MODULUS) + 1
    valid = (np.arange(Ss) >= n - 1)[None, :, None]
    return np.where(valid, hmod, 0)


def _global_indices(input_ids, hm2, ho2, hm3, ho3):
    """[B, S, 8] int32 row indices into the flattened [8*65536, 256] table."""
    h2 = _hash_ids_np(input_ids, hm2, ho2, 2)
    h3 = _hash_ids_np(input_ids, hm3, ho3, 3)
    hid = np.concatenate([h2, h3], axis=-1)          # [B, S, 8]
    gidx = hid + (np.arange(HEADS, dtype=np.int64) * VOCAB)[None, None, :]
    return gidx.astype(np.int32)


# ---------------------------------------------------------------- device program
def _emit_tile(nc, pools, cons, r, idx_ap, hid_dram_rows, hidh_dram, out_rows,
               b_prev, is_halo):
    """Emit one token tile (r=128 main tile, or r=2 halo).

    Returns the b-tile (boundary conv sources) for the next tile.
    """
    (pm, pmt, pg1, pg0, pscr, pstat, pb, ppsum_kv, ppsum_aux, ppsum_cv) = pools
    (wk_sb, wv_sb, w1_sb, w0_sb, w2p1_sb, s1_sb, s2_sb, id_sb, idb_sb,
     eps_sb, gb_sb) = cons

    # ---- gather memory rows: two halves of 4 heads each -> [r, 1024] f32
    # (one indirect DMA per head: HW consumes exactly one offset per
    #  partition row; multi-offset APs scramble on device)
    m_halves = []
    for half in range(2):
        m = pm.tile([P, 4 * HEAD_DIM], BF16, tag="m")
        for j in range(4):
            h = half * 4 + j
            nc.gpsimd.indirect_dma_start(
                out=m[:r, j * HEAD_DIM:(j + 1) * HEAD_DIM],
                out_offset=None,
                in_=nc.t_tables[:],
                in_offset=bass.IndirectOffsetOnAxis(
                    ap=idx_ap[:, h:h + 1], axis=0),
            )
        m_halves.append(m)

    # ---- PE transposes -> MT [128, r per k-slab] bf16  (lhsT layout)
    # MT[p, k*r + t] = memory[t, k*128 + p]
    mt = pmt.tile([P, KT * r], BF16, tag="mt")
    for q in range(4):
        aux = ppsum_aux.tile([P, 4 * r], BF16, tag="aux")
        for j in range(4):
            src = m_halves[q // 2][:r, (q % 2) * 4 * P + j * P:
                                       (q % 2) * 4 * P + (j + 1) * P]
            nc.tensor.transpose(
                aux[:, j * r:(j + 1) * r],
                src,
                idb_sb[:r, :r],
            )
        nc.scalar.copy(out=mt[:, q * 4 * r:(q + 1) * 4 * r], in_=aux[:])

    def gemm(w_sb):
        ph = [ppsum_kv.tile([P, 1024], F32, tag="pkv", name=f"pkv{h}")
              for h in range(2)]
        for half in range(2):
            for k in range(KT):
                lhs = mt[:, k * r:(k + 1) * r]
                for j in range(2):
                    col = half * 1024 + j * 512
                    nc.tensor.matmul(
                        ph[half][:r, j * 512:(j + 1) * 512],
                        lhsT=lhs,
                        rhs=w_sb[:, k * H + col: k * H + col + 512],
                        start=(k == 0),
                        stop=(k == KT - 1),
                    )
        return ph

    def sumsq(ph, tagc):
        sq = pstat.tile([P, 2], F32, tag="sq")
        for half in range(2):
            scr = pscr.tile([P, 1024], F32, tag="scr")
            nc.scalar.activation(
                out=scr[:r], in_=ph[half][:r],
                func=mybir.ActivationFunctionType.Square,
                accum_out=sq[:r, half:half + 1],
            )
        tot = pstat.tile([P, 1], F32, tag=tagc)
        nc.vector.tensor_reduce(out=tot[:r], in_=sq[:r], axis=mybir.AxisListType.X,
                                op=mybir.AluOpType.add)
        # tot = 1/sqrt(mean + eps) = exp(-0.5 * ln(mean + eps)); ln/exp live in
        # the same ACT table set as square/copy, so no table reloads.
        nc.scalar.activation(out=tot[:r], in_=tot[:r],
                             func=mybir.ActivationFunctionType.Ln,
                             scale=1.0 / H, bias=eps_sb[:r])
        nc.scalar.activation(out=tot[:r], in_=tot[:r],
                             func=mybir.ActivationFunctionType.Exp,
                             scale=-0.5)
        return tot

    # ---- key phase
    pk = gemm(wk_sb)
    rsq_k = sumsq(pk, "rsqk")
    dot2 = pstat.tile([P, 2], F32, tag="dot2")
    for half in range(2):
        hid = pm.tile([P, 1024], F32, tag="hid")
        src = hidh_dram if is_halo else hid_dram_rows
        nc.sync.dma_start(out=hid[:r], in_=src[:, half * 1024:(half + 1) * 1024])
        scr = pscr.tile([P, 1024], F32, tag="scr")
        nc.vector.scalar_tensor_tensor(
            out=scr[:r], in0=pk[half][:r], scalar=1.0, in1=hid[:r],
            op0=mybir.AluOpType.mult, op1=mybir.AluOpType.mult,
            accum_out=dot2[:r, half:half + 1],
        )
    dott = pstat.tile([P, 1], F32, tag="dott")
    nc.vector.tensor_reduce(out=dott[:r], in_=dot2[:r], axis=mybir.AxisListType.X,
                            op=mybir.AluOpType.add)
    nc.vector.tensor_mul(out=dott[:r], in0=dott[:r], in1=rsq_k[:r])
    # sigmoid(z) = 1 / (1 + exp(-z)); z = dott/sqrt(H) + GATE_BIAS
    sgate = pstat.tile([P, 1], F32, tag="sgate")
    nc.scalar.activation(out=sgate[:r], in_=dott[:r],
                         func=mybir.ActivationFunctionType.Exp,
                         scale=-1.0 / float(np.sqrt(H)), bias=gb_sb[:r])
    nc.vector.tensor_scalar_add(sgate[:r], sgate[:r], 1.0)
    nc.vector.reciprocal(out=sgate[:r], in_=sgate[:r])

    # ---- value phase
    pv = gemm(wv_sb)
    rsq_v = sumsq(pv, "rsqv")
    nc.vector.tensor_mul(out=sgate[:r], in0=sgate[:r], in1=rsq_v[:r])

    # g1 = (v*s)*w1bc (bf16), g0 likewise, t1 = (v*s)*w2p1 (f32)
    g1 = pg1.tile([P, H], BF16, tag="g1")
    g0 = pg0.tile([P, H], BF16, tag="g0")
    t1h = []
    for half in range(2):
        cs = slice(half * 1024, (half + 1) * 1024)
        nc.vector.scalar_tensor_tensor(
            out=g1[:r, cs], in0=pv[half][:r], scalar=sgate[:r], in1=w1_sb[:r, cs],
            op0=mybir.AluOpType.mult, op1=mybir.AluOpType.mult)
        nc.vector.scalar_tensor_tensor(
            out=g0[:r, cs], in0=pv[half][:r], scalar=sgate[:r], in1=w0_sb[:r, cs],
            op0=mybir.AluOpType.mult, op1=mybir.AluOpType.mult)
        if not is_halo:
            t1 = pscr.tile([P, 1024], F32, tag="scr")
            nc.vector.scalar_tensor_tensor(
                out=t1[:r], in0=pv[half][:r], scalar=sgate[:r], in1=w2p1_sb[:r, cs],
                op0=mybir.AluOpType.mult, op1=mybir.AluOpType.mult)
            t1h.append(t1)

    if (not is_halo) and getattr(nc, 'dbg', None) and nc.dbg and not nc.dbg.get('_done'):
        nc.dbg['_done'] = True
        nc.sync.dma_start(out=nc.dbg['dbg_m0'][:], in_=m_halves[0][:])
        nc.sync.dma_start(out=nc.dbg['dbg_m1'][:], in_=m_halves[1][:])
        nc.sync.dma_start(out=nc.dbg['dbg_g1'][:], in_=g1[:])
        nc.sync.dma_start(out=nc.dbg['dbg_sgate'][:], in_=sgate[:])
        nc.sync.dma_start(out=nc.dbg['dbg_rsqv'][:], in_=rsq_v[:])
        nc.sync.dma_start(out=nc.dbg['dbg_mt'][:], in_=mt[:])
        for half in range(2):
            nc.sync.dma_start(out=nc.dbg['dbg_t1'][:, half * 1024:(half + 1) * 1024],
                              in_=t1h[half][:])

    # ---- boundary sources for the NEXT tile: b[0] = g0[r-2], b[1] = g0[r-1],
    #      b[0] += g1[r-1]
    b_next = pb.tile([2, H], BF16, tag="b")
    nc.sync.dma_start(out=b_next[0:2, :], in_=g0[r - 2:r, :])
    nc.gpsimd.dma_start(out=b_next[0:1, :], in_=g1[r - 1:r, :],
                        accum_op=mybir.AluOpType.add)

    if is_halo:
        return b_next

    # ---- conv shifts + final add + store
    for half in range(2):
        for j in range(2):
            cs = slice(half * 1024 + j * 512, half * 1024 + (j + 1) * 512)
            pcv = ppsum_cv.tile([P, 512], F32, tag="pcv")
            nc.tensor.matmul(pcv[:], lhsT=s1_sb[:], rhs=g1[:, cs],
                             start=True, stop=False)
            nc.tensor.matmul(pcv[:], lhsT=s2_sb[:], rhs=g0[:, cs],
                             start=False, stop=True)
            nc.vector.tensor_add(out=pcv[0:2, :], in0=pcv[0:2, :],
                                 in1=b_prev[0:2, cs])
            nc.vector.tensor_add(out=t1h[half][:, j * 512:(j + 1) * 512],
                                 in0=t1h[half][:, j * 512:(j + 1) * 512],
                                 in1=pcv[:])
        nc.sync.dma_start(out=out_rows[:, half * 1024:(half + 1) * 1024],
                          in_=t1h[half][:])
    return b_next


def build_program(nt=NT, table_rows=HEADS * VOCAB, debug=False):
    nc = bacc.Bacc(None, target_bir_lowering=False)
    tok = nt * P

    t_tables = nc.dram_tensor("tables", [table_rows, HEAD_DIM], BF16,
                              kind="ExternalInput")
    t_hidden = nc.dram_tensor("hidden", [tok, H], F32, kind="ExternalInput")
    t_hidh = nc.dram_tensor("hidh", [2, H], F32, kind="ExternalInput")
    t_idx = nc.dram_tensor("idx", [P, nt, HEADS], I32, kind="ExternalInput")
    t_idxh = nc.dram_tensor("idxh", [2, HEADS], I32, kind="ExternalInput")
    t_wk = nc.dram_tensor("wkt", [P, KT * H], BF16, kind="ExternalInput")
    t_wv = nc.dram_tensor("wvt", [P, KT * H], BF16, kind="ExternalInput")
    t_w1 = nc.dram_tensor("w1bc", [P, H], BF16, kind="ExternalInput")
    t_w0 = nc.dram_tensor("w0bc", [P, H], BF16, kind="ExternalInput")
    t_w2p1 = nc.dram_tensor("w2p1bc", [P, H], F32, kind="ExternalInput")
    t_s1 = nc.dram_tensor("s1", [P, P], BF16, kind="ExternalInput")
    t_s2 = nc.dram_tensor("s2", [P, P], BF16, kind="ExternalInput")
    t_id = nc.dram_tensor("ident", [P, P], F32, kind="ExternalInput")
    t_idb = nc.dram_tensor("identb", [P, P], BF16, kind="ExternalInput")
    t_out = nc.dram_tensor("out", [tok, H], F32, kind="ExternalOutput")
    nc.t_tables = t_tables
    nc.dbg = {}
    if debug:
        for nm, shape, dt_ in (("dbg_t1", [P, H], F32), ("dbg_g1", [P, H], BF16),
                               ("dbg_sgate", [P, 1], F32),
                               ("dbg_mt", [P, KT * P], BF16), ("dbg_rsqv", [P, 1], F32),
                               ("dbg_m0", [P, 1024], F32), ("dbg_m1", [P, 1024], F32)):
            nc.dbg[nm] = nc.dram_tensor(nm, shape, dt_, kind="ExternalOutput")

    with tile.TileContext(nc) as tc:
        with (
            tc.tile_pool(name="const", bufs=1) as pconst,
            tc.tile_pool(name="pm", bufs=3) as pm,
            tc.tile_pool(name="pmt", bufs=2) as pmt,
            tc.tile_pool(name="pg1", bufs=2) as pg1,
            tc.tile_pool(name="pg0", bufs=2) as pg0,
            tc.tile_pool(name="pscr", bufs=2) as pscr,
            tc.tile_pool(name="pstat", bufs=2) as pstat,
            tc.tile_pool(name="pb", bufs=2) as pb,
            tc.tile_pool(name="ppsum_kv", bufs=3, space="PSUM") as ppsum_kv,
            tc.tile_pool(name="ppsum_aux", bufs=1, space="PSUM") as ppsum_aux,
            tc.tile_pool(name="ppsum_cv", bufs=1, space="PSUM") as ppsum_cv,
        ):
            # resident constants
            wk_sb = pconst.tile([P, KT * H], BF16)
            wv_sb = pconst.tile([P, KT * H], BF16)
            w1_sb = pconst.tile([P, H], BF16)
            w0_sb = pconst.tile([P, H], BF16)
            w2p1_sb = pconst.tile([P, H], F32)
            s1_sb = pconst.tile([P, P], BF16)
            s2_sb = pconst.tile([P, P], BF16)
            id_sb = pconst.tile([P, P], F32)
            idb_sb = pconst.tile([P, P], BF16)
            eps_sb = pconst.tile([P, 1], F32)
            gb_sb = pconst.tile([P, 1], F32)
            nc.vector.memset(eps_sb[:], EPS)
            nc.vector.memset(gb_sb[:], -GATE_BIAS)
            idx_sb = pconst.tile([P, nt, HEADS], I32)
            idxh_sb = pconst.tile([2, HEADS], I32)
            for dst, src in ((w1_sb, t_w1),
                             (w0_sb, t_w0), (w2p1_sb, t_w2p1), (s1_sb, t_s1),
                             (s2_sb, t_s2), (id_sb, t_id), (idb_sb, t_idb),
                             (idx_sb, t_idx),
                             (idxh_sb, t_idxh)):
                nc.sync.dma_start(out=dst[:], in_=src[:])
            # per-k-slab weight loads so the first matmuls start after slab 0
            for k in range(KT):
                cs = slice(k * H, (k + 1) * H)
                nc.sync.dma_start(out=wk_sb[:, cs], in_=t_wk[:, cs])
                nc.sync.dma_start(out=wv_sb[:, cs], in_=t_wv[:, cs])

            pools = (pm, pmt, pg1, pg0, pscr, pstat, pb, ppsum_kv, ppsum_aux,
                     ppsum_cv)
            cons = (wk_sb, wv_sb, w1_sb, w0_sb, w2p1_sb, s1_sb, s2_sb, id_sb,
                    idb_sb, eps_sb, gb_sb)

            # halo: 2 leading tokens (zero rows when idxh = 0)
            b_prev = _emit_tile(nc, pools, cons, 2, idxh_sb[:], None,
                                t_hidh[:], None, None, True)
            for i in range(nt):
                rows = slice(i * P, (i + 1) * P)
                b_prev = _emit_tile(
                    nc, pools, cons, P, idx_sb[:, i, :], t_hidden[rows, :],
                    None, t_out[rows, :], b_prev, False)

    nc.compile()
    return nc


# ---------------------------------------------------------------- host wrapper
_PROGRAM = None


def _get_program():
    global _PROGRAM
    if _PROGRAM is None:
        _PROGRAM = build_program()
    return _PROGRAM


def kernel(hidden_states, input_ids, tables, Wk, Wv, key_norm_w, value_norm_w,
           conv_w, hm2, ho2, hm3, ho3):
    hidden_states = np.asarray(hidden_states, dtype=np.float32)
    input_ids = np.asarray(input_ids, dtype=np.int64)
    tables = np.asarray(tables, dtype=np.float32)
    Wk = np.asarray(Wk, dtype=np.float32)
    Wv = np.asarray(Wv, dtype=np.float32)
    key_norm_w = np.asarray(key_norm_w, dtype=np.float32)
    value_norm_w = np.asarray(value_norm_w, dtype=np.float32)
    conv_w = np.asarray(conv_w, dtype=np.float32)

    gidx = _global_indices(input_ids, np.asarray(hm2), np.asarray(ho2),
                           np.asarray(hm3), np.asarray(ho3))   # [B,S,8] i32
    gidx_flat = gidx.reshape(B * S, HEADS)
    tables_flat = None  # set below after bf16 cast

    # fold key_norm_w into hidden (gate dot), value_norm_w into conv weights
    if not np.all(key_norm_w == 1.0):
        hidden_states = hidden_states * key_norm_w[None, None, :]
    hid_flat = np.ascontiguousarray(hidden_states.reshape(B * S, H))

    w0 = conv_w[:, 0] * value_norm_w
    w1 = conv_w[:, 1] * value_norm_w
    w2p1 = (1.0 + conv_w[:, 2]) * value_norm_w

    import ml_dtypes
    bf = ml_dtypes.bfloat16
    tables_flat = np.ascontiguousarray(
        tables.reshape(HEADS * VOCAB, HEAD_DIM)).astype(bf)

    def kslab_bf(W):
        A = np.ascontiguousarray(W.T).reshape(KT, P, H).transpose(1, 0, 2)
        return np.ascontiguousarray(A.reshape(P, KT * H)).astype(bf)

    wk_host = kslab_bf(Wk)
    wv_host = kslab_bf(Wv)
    w1bc = np.ascontiguousarray(np.broadcast_to(w1, (P, H))).astype(bf)
    w0bc = np.ascontiguousarray(np.broadcast_to(w0, (P, H))).astype(bf)
    w2p1bc = np.ascontiguousarray(np.broadcast_to(w2p1, (P, H))).astype(np.float32)
    s1 = np.eye(P, k=1).astype(bf)
    s2 = np.eye(P, k=2).astype(bf)
    ident = np.eye(P, dtype=np.float32)
    identb = np.eye(P).astype(bf)

    in_maps = []
    for r in range(N_CORES):
        t0 = r * TOK_PER_CORE
        idx_core = gidx_flat[t0:t0 + TOK_PER_CORE]          # [2048, 8]
        idx_host = np.ascontiguousarray(
            idx_core.reshape(NT, P, HEADS).transpose(1, 0, 2))  # [128, NT, 8]
        if t0 % S == 0:
            idxh = np.zeros((2, HEADS), np.int32)
            hidh = np.zeros((2, H), np.float32)
        else:
            idxh = np.ascontiguousarray(gidx_flat[t0 - 2:t0])
            hidh = np.ascontiguousarray(hid_flat[t0 - 2:t0])
        in_maps.append({
            "tables": tables_flat,
            "hidden": np.ascontiguousarray(hid_flat[t0:t0 + TOK_PER_CORE]),
            "hidh": hidh,
            "idx": idx_host,
            "idxh": idxh,
            "wkt": wk_host,
            "wvt": wv_host,
            "w1bc": w1bc,
            "w0bc": w0bc,
            "w2p1bc": w2p1bc,
            "s1": s1,
            "s2": s2,
            "ident": ident,
            "identb": identb,
        })

    nc = _get_program()
    res = run_bass_kernel_spmd(nc, in_maps, list(range(N_CORES)))
    out = np.empty((B * S, H), np.float32)
    for r in range(N_CORES):
        out[r * TOK_PER_CORE:(r + 1) * TOK_PER_CORE] = res.results[r]["out"]
    return out.reshape(B, S, H)



# revision 20
# speedup vs baseline: 1.4201x; 1.4201x over previous
"""EngramModule kernel for Trainium2 (8 NeuronCores, SPMD data-parallel).

v2 architecture (fp8 DoubleRow 3-term GEMMs + transpose-gather):

Per token t (feature dim H=2048):
  idx[t, h]   = hash of n-gram ending at t (8 heads; computed on host)
  memory[t]   = concat_h tables[h, idx[t, h]]
  key_raw     = memory @ Wk.T ; value_raw = memory @ Wv.T
  gate        = sigmoid(dot(hidden, key_raw)/(sqrt(H)*rms_k) - 4)
  g[t]        = gate * value_raw / rms_v           (value_norm folded in conv w)
  out[t]      = g[t]*(1+w2) + w1*g[t-1] + w0*g[t-2]

Device strategy per core (2048 tokens, 16 tiles of 128):
 - Tables are compacted on host per (core, head) to <=2048 unique rows and
   packed as fp8 e4m3 (hi, lo) byte-interleaved 512B rows. One dma_gather
   with transpose=True per (head, segment) delivers memory ALREADY in lhsT
   layout [k-dim on partitions, tokens on free], with (hi, lo) as the two
   bytes of each 16-bit transpose unit -> directly usable as DoubleRow
   operand slots.
 - GEMMs run as 3-term compensated fp8 DoubleRow matmuls (error ~0.1%):
     (Mhi 2-slab)x(Whi 2-slab) + (Mlo)x(Whi) + (Mhi)x(Wlo)
   at 0.25 PE-cycles per output column per 128-contraction.
 - Phase 1 (key): per tile accumulate k in PSUM, ACT-square sumsq + DVE dot
   with bf16 hidden, Newton rsqrt, one ACT Sigmoid -> per-token gate.
 - Phase 2 (value): v in PSUM, sumsq -> rsq_v, g = ACT copy(v * s) bf16;
   g1/g0 cast to fp8; conv = one fp8-DR matmul (shift-pair slots) + one
   fp8-DR boundary matmul per 512-chunk; out = ACT copy(conv psum) + g2.
 - Conv boundary rows for tile 0 are computed EXACTLY on host (2 tokens).
 - Output written bf16, upcast on host.
Only ACT funcs {Square, Sigmoid, Copy} are used -> single act table set,
zero LoadActFuncSet reloads.
"""

import sys

import numpy as np

try:
    import concourse.bass as bass  # noqa: F401
except ImportError:
    sys.path.insert(0, "/opt/trn_rl_repo")

import ml_dtypes

import concourse.bacc as bacc
import concourse.bass as bass
import concourse.tile as tile
from concourse import mybir
from concourse.bass_utils import run_bass_kernel_spmd

E4 = ml_dtypes.float8_e4m3fn
BFNP = ml_dtypes.bfloat16
F32 = mybir.dt.float32
BF16 = mybir.dt.bfloat16
FP8 = mybir.dt.float8e4
I16 = mybir.dt.int16
I32 = mybir.dt.int32
ALU = mybir.AluOpType
AF = mybir.ActivationFunctionType
DR = mybir.MatmulPerfMode.DoubleRow

P = 128
H = 2048
HEADS = 8
HEAD_DIM = 256
VOCAB = 65536
MODULUS = VOCAB - 1
EPS = 1e-6
GATE_BIAS = -4.0
N_CORES = 8
B, S = 4, 4096
TOK = (B * S) // N_CORES        # 2048 tokens per core
NT = TOK // P                   # 16 tiles
CROWS = 2048                    # compact table rows per (core, head)
# gather segments (tokens); each <=1024 so one gather fits the default
# 1024-descriptor SWDGE ring. First segment small so tile 0 starts early.
SEGS = (256, 896, 896)
SEG_OFF = (0, 256, 1152)
SG = 128.0                      # fp8 scale for g1/g0 (conv operands)
FMAX = 64.0                     # fp8 operand absmax (PSUM partial < ~5.5e4)
NCH = H // 512                  # 4 col chunks of 512


# ---------------------------------------------------------------- host hashing
def _hash_ids_np(ids, mult, off, n):
    """Exact replica of the reference _hash_ids in numpy (wrapping int64)."""
    Bb, Ss = ids.shape
    nh = mult.shape[0]
    ids_u = ids.astype(np.uint64)
    mult_u = mult.astype(np.uint64)
    off_u = off.astype(np.uint64)
    mix = np.zeros((Bb, Ss, nh), dtype=np.uint64)
    for p in range(n):
        shift = n - 1 - p
        tok = np.zeros_like(ids_u)
        if shift > 0:
            tok[:, shift:] = ids_u[:, : Ss - shift]
        else:
            tok = ids_u
        mix ^= tok[:, :, None] * mult_u[None, None, :, p]
    h = (mix + off_u[None, None, :]).view(np.int64)
    hmod = np.remainder(h, MODULUS) + 1
    valid = (np.arange(Ss) >= n - 1)[None, :, None]
    return np.where(valid, hmod, 0)


def _global_indices(input_ids, hm2, ho2, hm3, ho3):
    """[B, S, 8] int32 row indices into the flattened [8*65536, 256] table."""
    h2 = _hash_ids_np(input_ids, hm2, ho2, 2)
    h3 = _hash_ids_np(input_ids, hm3, ho3, 3)
    hid = np.concatenate([h2, h3], axis=-1)          # [B, S, 8]
    gidx = hid + (np.arange(HEADS, dtype=np.int64) * VOCAB)[None, None, :]
    return gidx.astype(np.int32)


def _wrap_idx(inv, n_tok):
    """int16 idx tile [128, n_tok//16]: slot i lives at [i%16, i//16]."""
    t = np.zeros((16, n_tok // 16), np.int16)
    t[np.arange(n_tok) % 16, np.arange(n_tok) // 16] = inv.astype(np.int16)
    return np.ascontiguousarray(np.tile(t, (8, 1)))


# ---------------------------------------------------------------- device program
def build_program():
    nc = bacc.Bacc(None, target_bir_lowering=False)

    t_ctab = [nc.dram_tensor(f"ctab{h}", [CROWS, 512], FP8, kind="ExternalInput")
              for h in range(HEADS)]
    t_idx = [[nc.dram_tensor(f"idx{s}_{h}", [P, SEGS[s] // 16], I16,
                             kind="ExternalInput") for h in range(HEADS)]
             for s in range(len(SEGS))]
    t_wkhi = nc.dram_tensor("wkhi", [P, HEADS, 2, H], FP8, kind="ExternalInput")
    t_wklo = nc.dram_tensor("wklo", [P, HEADS, 2, H], FP8, kind="ExternalInput")
    t_wvhi = nc.dram_tensor("wvhi", [P, HEADS, 2, H], FP8, kind="ExternalInput")
    t_wvlo = nc.dram_tensor("wvlo", [P, HEADS, 2, H], FP8, kind="ExternalInput")
    t_hid = nc.dram_tensor("hid", [TOK, H], BF16, kind="ExternalInput")
    t_w2p1 = nc.dram_tensor("w2p1bc", [P, H], BF16, kind="ExternalInput")
    t_w1s = nc.dram_tensor("w1sbc", [P, H], BF16, kind="ExternalInput")
    t_w0s = nc.dram_tensor("w0sbc", [P, H], BF16, kind="ExternalInput")
    t_s12 = nc.dram_tensor("s12", [P, 2, P], FP8, kind="ExternalInput")
    t_eb = nc.dram_tensor("eb", [P, 2, P], FP8, kind="ExternalInput")
    t_bh = nc.dram_tensor("bhost", [4, H], FP8, kind="ExternalInput")
    t_out = nc.dram_tensor("out", [TOK, H], BF16, kind="ExternalOutput")

    # scalar constants (host-computed, folded scales)
    t_consts = nc.dram_tensor("consts", [P, 4], F32, kind="ExternalInput")
    # consts columns: 0 = eps_k' = (sM*sWk)^2 * H * EPS
    #                 1 = eps_v' = (sM*sWv)^2 * H * EPS
    #                 2 = sqrt(H)
    #                 3 = GATE_BIAS

    with tile.TileContext(nc) as tc:
        with (
            tc.tile_pool(name="pconst", bufs=1) as pc,
            tc.tile_pool(name="pM", bufs=1) as pM,
            tc.tile_pool(name="pWvhi", bufs=1) as pWvhi,
            tc.tile_pool(name="pstat", bufs=4) as pst,
            tc.tile_pool(name="pscr", bufs=2) as pscr,
            tc.tile_pool(name="pbig", bufs=3, space="PSUM") as pbig,
        ):
            # ---- small consts
            consts = pc.tile([P, 4], F32)
            nc.sync.dma_start(out=consts[:], in_=t_consts[:])
            s12_sb = pc.tile([P, 2, P], FP8)
            nc.sync.dma_start(out=s12_sb[:], in_=t_s12[:])
            eb_sb = pc.tile([P, 2, P], FP8)
            nc.sync.dma_start(out=eb_sb[:], in_=t_eb[:])
            idx_sb = [[pc.tile([P, SEGS[s] // 16], I16, name=f"ix{s}_{h}")
                       for h in range(HEADS)] for s in range(len(SEGS))]
            for s in range(len(SEGS)):
                for h in range(HEADS):
                    nc.sync.dma_start(out=idx_sb[s][h][:], in_=t_idx[s][h][:])
            sg_all = pc.tile([P, NT], F32)      # per-tile gate scalars

            # ---- gathers (segment 0 small so tile 0 can start early)
            mseg = [[None] * HEADS for _ in SEGS]
            for s in range(len(SEGS)):
                for h in range(HEADS):
                    m = pM.tile([P, 4 * SEGS[s]], FP8, name=f"m{s}_{h}")
                    nc.gpsimd.dma_gather(
                        out_ap=m[:].rearrange("p (f t) -> p f t", f=4),
                        in_ap=t_ctab[h][:],
                        idxs_ap=idx_sb[s][h][:],
                        num_idxs=SEGS[s], num_idxs_reg=SEGS[s],
                        elem_size=512, transpose=True)
                    mseg[s][h] = m

            def m_slabs(h, i):
                """(lhsT_hi, lhsT_lo) [p, c(2), t(128)] for tile i, head h."""
                tok0 = i * P
                s = max(x for x in range(len(SEGS)) if SEG_OFF[x] <= tok0)
                m, t0, ts = mseg[s][h], tok0 - SEG_OFF[s], SEGS[s]
                ctj = m[:].rearrange("p (c t j) -> p c t j", c=2, t=ts, j=2)
                return (ctj[:, :, t0:t0 + P, 0], ctj[:, :, t0:t0 + P, 1])

            def gemm_tile(i, whi_sb, wlo_sb, ph):
                """3-term fp8 DR GEMM for tile i into psum halves ph[0|1]."""
                for ch in range(NCH):
                    pt = ph[ch // 2]
                    cs = slice((ch % 2) * 512, (ch % 2) * 512 + 512)
                    ws = slice(ch * 512, ch * 512 + 512)
                    for h in range(HEADS):
                        hi, lo = m_slabs(h, i)
                        whi = whi_sb[:, h, :, ws]
                        wlo = wlo_sb[:, h, :, ws]
                        st = (h == 0)
                        nc.tensor.matmul(pt[:, cs], lhsT=hi, rhs=whi,
                                         start=st, stop=False, perf_mode=DR)
                        nc.tensor.matmul(pt[:, cs], lhsT=lo, rhs=whi,
                                         start=False, stop=False, perf_mode=DR)
                        nc.tensor.matmul(pt[:, cs], lhsT=hi, rhs=wlo,
                                         start=False, stop=(h == HEADS - 1),
                                         perf_mode=DR)

            def newton_rsqrt(u):
                """In-place u <- 1/sqrt(u) via bit-seed + 2 Newton iters."""
                y = pst.tile([P, 1], F32, tag="ny")
                yi = y[:].bitcast(I32)
                nc.vector.tensor_scalar(out=yi, in0=u[:].bitcast(I32), scalar1=1,
                                        scalar2=None, op0=ALU.logical_shift_right)
                nc.vector.tensor_scalar(out=yi, in0=yi, scalar1=-1,
                                        scalar2=0x5F3759DF, op0=ALU.mult,
                                        op1=ALU.add)
                t2 = pst.tile([P, 1], F32, tag="nt")
                for _ in range(2):
                    nc.vector.tensor_mul(out=t2[:], in0=y[:], in1=y[:])
                    nc.vector.tensor_mul(out=t2[:], in0=t2[:], in1=u[:])
                    nc.vector.tensor_scalar(out=t2[:], in0=t2[:], scalar1=-0.5,
                                            scalar2=1.5, op0=ALU.mult, op1=ALU.add)
                    nc.vector.tensor_mul(out=y[:], in0=y[:], in1=t2[:])
                return y

            # ================= PHASE 1: key =================
            # prefetch Wv_hi into non-overlapping outer-scope space (Pool queue)
            wvhi_sb = pWvhi.tile([P, HEADS, 2, H], FP8)
            for h in range(HEADS):
                nc.gpsimd.dma_start(out=wvhi_sb[:, h], in_=t_wvhi[:, h])

            with (
                tc.tile_pool(name="pWk", bufs=1) as pWk,
                tc.tile_pool(name="phid", bufs=3) as phid,
            ):
                wkhi_sb = pWk.tile([P, HEADS, 2, H], FP8)
                wklo_sb = pWk.tile([P, HEADS, 2, H], FP8)
                for h in range(HEADS):
                    nc.sync.dma_start(out=wkhi_sb[:, h], in_=t_wkhi[:, h])
                    nc.sync.dma_start(out=wklo_sb[:, h], in_=t_wklo[:, h])

                for i in range(NT):
                    hid_sb = phid.tile([P, H], BF16, tag="hid")
                    nc.sync.dma_start(out=hid_sb[:],
                                      in_=t_hid[i * P:(i + 1) * P, :])
                    ph = [pbig.tile([P, 1024], F32, tag="ps", name=f"k{i}_{x}")
                          for x in range(2)]
                    gemm_tile(i, wkhi_sb, wklo_sb, ph)
                    # sumsq + dot per half
                    sq2 = pst.tile([P, 2], F32, tag="sq2")
                    dt2 = pst.tile([P, 2], F32, tag="dt2")
                    scr = pscr.tile([P, 1024], F32, tag="scr")
                    for x in range(2):
                        nc.scalar.activation(out=scr[:], in_=ph[x][:],
                                             func=AF.Square,
                                             accum_out=sq2[:, x:x + 1])
                        nc.vector.scalar_tensor_tensor(
                            out=scr[:], in0=ph[x][:], scalar=1.0,
                            in1=hid_sb[:, x * 1024:(x + 1) * 1024],
                            op0=ALU.mult, op1=ALU.mult,
                            accum_out=dt2[:, x:x + 1])
                    u = pst.tile([P, 1], F32, tag="u")
                    nc.vector.tensor_reduce(out=u[:], in_=sq2[:],
                                            axis=mybir.AxisListType.X,
                                            op=ALU.add)
                    # u = sumsq + eps_k'  (rsq' = 1/sqrt(u) folds the /sqrt(H))
                    nc.vector.tensor_scalar(out=u[:], in0=u[:],
                                            scalar1=consts[:, 0:1], scalar2=None,
                                            op0=ALU.add)
                    rsq = newton_rsqrt(u)
                    dot = pst.tile([P, 1], F32, tag="dot")
                    nc.vector.tensor_reduce(out=dot[:], in_=dt2[:],
                                            axis=mybir.AxisListType.X,
                                            op=ALU.add)
                    # gate = sigmoid(dot * rsq' - 4)
                    nc.scalar.activation(out=sg_all[:, i:i + 1], in_=dot[:],
                                         func=AF.Sigmoid, scale=rsq[:],
                                         bias=consts[:, 3:4])

            # ================= PHASE 2: value =================
            with (
                tc.tile_pool(name="pWvlo", bufs=1) as pWvlo,
                tc.tile_pool(name="pg", bufs=2) as pg,
                tc.tile_pool(name="pout", bufs=2) as pout,
                tc.tile_pool(name="pb", bufs=1) as pb,
                tc.tile_pool(name="pconv", bufs=2, space="PSUM") as pconv,
            ):
                wvlo_sb = pWvlo.tile([P, HEADS, 2, H], FP8)
                for h in range(HEADS):
                    nc.gpsimd.dma_start(out=wvlo_sb[:, h], in_=t_wvlo[:, h])
                w2p1_sb = pWvlo.tile([P, H], BF16)
                w1s_sb = pWvlo.tile([P, H], BF16)
                w0s_sb = pWvlo.tile([P, H], BF16)
                nc.sync.dma_start(out=w2p1_sb[:], in_=t_w2p1[:])
                nc.sync.dma_start(out=w1s_sb[:], in_=t_w1s[:])
                nc.sync.dma_start(out=w0s_sb[:], in_=t_w0s[:])

                # boundary ping-pong tiles (fixed, fully memset once so reads
                # of untouched rows are well-defined); tile 0's rows from host
                b_tiles = [pb.tile([P, 2, H], FP8, name=f"b{x}")
                           for x in range(2)]
                nc.vector.memset(b_tiles[0][:], 0.0)
                nc.vector.memset(b_tiles[1][:], 0.0)
                nc.sync.dma_start(out=b_tiles[0][0:4, 0, :], in_=t_bh[:])

                for i in range(NT):
                    b_prev = b_tiles[i % 2]
                    ph = [pbig.tile([P, 1024], F32, tag="ps", name=f"v{i}_{x}")
                          for x in range(2)]
                    gemm_tile(i, wvhi_sb, wvlo_sb, ph)
                    sq2 = pst.tile([P, 2], F32, tag="sq2")
                    scr = pscr.tile([P, 1024], F32, tag="scr")
                    for x in range(2):
                        nc.scalar.activation(out=scr[:], in_=ph[x][:],
                                             func=AF.Square,
                                             accum_out=sq2[:, x:x + 1])
                    u = pst.tile([P, 1], F32, tag="u")
                    nc.vector.tensor_reduce(out=u[:], in_=sq2[:],
                                            axis=mybir.AxisListType.X,
                                            op=ALU.add)
                    nc.vector.tensor_scalar(out=u[:], in0=u[:],
                                            scalar1=consts[:, 1:2], scalar2=None,
                                            op0=ALU.add)
                    rsq = newton_rsqrt(u)
                    # s_final = rsq * sqrt(H) * gate
                    sfin = pst.tile([P, 1], F32, tag="sfin")
                    nc.vector.scalar_tensor_tensor(
                        out=sfin[:], in0=rsq[:], scalar=consts[:, 2:3],
                        in1=sg_all[:, i:i + 1], op0=ALU.mult, op1=ALU.mult)
                    # g = v * s_final   (bf16)
                    g = pg.tile([P, H], BF16, tag="g")
                    for x in range(2):
                        nc.scalar.activation(
                            out=g[:, x * 1024:(x + 1) * 1024], in_=ph[x][:],
                            func=AF.Copy, scale=sfin[:])
                    # g2 bf16; g1/g0 -> fp8 slots of g10
                    g2 = pg.tile([P, H], BF16, tag="g2")
                    nc.vector.tensor_mul(out=g2[:], in0=g[:], in1=w2p1_sb[:])
                    g10 = pg.tile([P, 2, H], FP8, tag="g10")
                    nc.vector.tensor_mul(out=g10[:, 0, :], in0=g[:], in1=w1s_sb[:])
                    nc.vector.tensor_mul(out=g10[:, 1, :], in0=g[:], in1=w0s_sb[:])

                    # conv psum per 512-chunk: shifts (DR) + boundary (DR)
                    out_sb = pout.tile([P, H], BF16, tag="out")
                    for ch in range(NCH):
                        cs = slice(ch * 512, (ch + 1) * 512)
                        pcv = pconv.tile([P, 512], F32, tag="pcv")
                        nc.tensor.matmul(pcv[:], lhsT=s12_sb[:],
                                         rhs=g10[:, :, cs],
                                         start=True, stop=False, perf_mode=DR)
                        nc.tensor.matmul(pcv[:], lhsT=eb_sb[:],
                                         rhs=b_prev[:, :, cs],
                                         start=False, stop=True, perf_mode=DR)
                        nc.scalar.activation(out=out_sb[:, cs], in_=pcv[:],
                                             func=AF.Copy)
                    nc.vector.tensor_add(out=out_sb[:], in0=out_sb[:], in1=g2[:])
                    nc.sync.dma_start(out=t_out[i * P:(i + 1) * P, :],
                                      in_=out_sb[:])

                    # boundary rows for next tile from g10 tails
                    if i < NT - 1:
                        b_next = b_tiles[(i + 1) % 2]
                        nc.sync.dma_start(out=b_next[0:1, 0, :],
                                          in_=g10[127:128, 0, :])
                        nc.sync.dma_start(out=b_next[1:3, 0, :],
                                          in_=g10[126:128, 1, :])

    nc.compile()
    return nc


# ---------------------------------------------------------------- host wrapper
_PROGRAM = None


def _get_program():
    global _PROGRAM
    if _PROGRAM is None:
        _PROGRAM = build_program()
    return _PROGRAM


def kernel(hidden_states, input_ids, tables, Wk, Wv, key_norm_w, value_norm_w,
           conv_w, hm2, ho2, hm3, ho3):
    hidden_states = np.asarray(hidden_states, dtype=np.float32)
    input_ids = np.asarray(input_ids, dtype=np.int64)
    tables = np.asarray(tables, dtype=np.float32)
    Wk = np.asarray(Wk, dtype=np.float32)
    Wv = np.asarray(Wv, dtype=np.float32)
    key_norm_w = np.asarray(key_norm_w, dtype=np.float32)
    value_norm_w = np.asarray(value_norm_w, dtype=np.float32)
    conv_w = np.asarray(conv_w, dtype=np.float32)

    gidx = _global_indices(input_ids, np.asarray(hm2), np.asarray(ho2),
                           np.asarray(hm3), np.asarray(ho3))   # [B,S,8] i32
    hid_local = (gidx % VOCAB).reshape(B * S, HEADS)           # per-head rows

    # fold key_norm into hidden; fp8 scales
    hid_flat = (hidden_states.reshape(B * S, H) * key_norm_w[None, :])
    hid_bf = hid_flat.astype(BFNP)

    sM = FMAX / max(np.abs(tables).max(), 1e-30)
    sWk = FMAX / max(np.abs(Wk).max(), 1e-30)
    sWv = FMAX / max(np.abs(Wv).max(), 1e-30)

    def split_fp8(x, s):
        hi = (x * s).astype(E4)
        lo = ((x * s) - hi.astype(np.float32)).astype(E4)
        return hi, lo

    def w_layout(W, s):
        # [p, head, c, n] with W.T[k, n] = W[n, k]; k = h*256 + c*128 + p
        hi, lo = split_fp8(np.ascontiguousarray(W.T), s)   # [k, n]
        def lay(a):
            return np.ascontiguousarray(
                a.reshape(HEADS, 2, P, H).transpose(2, 0, 1, 3))
        return lay(hi), lay(lo)

    wkhi, wklo = w_layout(Wk, sWk)
    wvhi, wvlo = w_layout(Wv, sWv)

    # conv weight foldings (value_norm + fp8 g-scale)
    w0 = conv_w[:, 0] * value_norm_w
    w1 = conv_w[:, 1] * value_norm_w
    w2p1 = (1.0 + conv_w[:, 2]) * value_norm_w
    w2p1bc = np.ascontiguousarray(np.broadcast_to(w2p1, (P, H))).astype(BFNP)
    w1sbc = np.ascontiguousarray(np.broadcast_to(w1 * SG, (P, H))).astype(BFNP)
    w0sbc = np.ascontiguousarray(np.broadcast_to(w0 * SG, (P, H))).astype(BFNP)

    # shift-pair (s1, s2) and boundary lhsT matrices, scaled by 1/SG
    inv = np.float32(1.0 / SG)
    s12 = np.zeros((P, 2, P), E4)
    s12[:, 0, :] = (np.eye(P, k=1, dtype=np.float32) * inv).astype(E4)
    s12[:, 1, :] = (np.eye(P, k=2, dtype=np.float32) * inv).astype(E4)
    eb = np.zeros((P, 2, P), E4)
    # b_pad rows: 0 -> out0 (g1[127]), 1 -> out0 (g0[126]), 2 -> out1 (g0[127])
    eb[0, 0, 0] = E4(inv)
    eb[1, 0, 0] = E4(inv)
    eb[2, 0, 1] = E4(inv)

    consts = np.zeros((P, 4), np.float32)
    consts[:, 0] = (sM * sWk) ** 2 * H * EPS
    consts[:, 1] = (sM * sWv) ** 2 * H * EPS
    consts[:, 2] = np.sqrt(np.float32(H))
    consts[:, 3] = GATE_BIAS

    # exact host reference for the 2 boundary tokens of each core
    def host_gated(trange):
        """gated[t] rows (f64->f32) for global token indices trange."""
        out = np.zeros((len(trange), H), np.float32)
        tabs = tables.astype(np.float64)
        for j, t in enumerate(trange):
            rows = [tabs[h, hid_local[t, h]] for h in range(HEADS)]
            mem = np.concatenate(rows)                     # [2048]
            kr = mem @ Wk.T.astype(np.float64)
            vr = mem @ Wv.T.astype(np.float64)
            rk = 1.0 / np.sqrt(np.mean(kr ** 2) + EPS)
            rv = 1.0 / np.sqrt(np.mean(vr ** 2) + EPS)
            z = float(hid_flat[t].astype(np.float64) @ (kr * rk)) / np.sqrt(H) \
                + GATE_BIAS
            gate = 1.0 / (1.0 + np.exp(-z))
            out[j] = (gate * (vr * rv) * value_norm_w).astype(np.float32)
        return out

    in_maps = []
    for r in range(N_CORES):
        t0 = r * TOK
        idx_core = hid_local[t0:t0 + TOK]                  # [2048, 8]
        in_map = {}
        for h in range(HEADS):
            uniq, invmap = np.unique(idx_core[:, h], return_inverse=True)
            rows = tables[h, uniq]                         # [n_u, 256]
            hi, lo = split_fp8(rows, sM)
            packed = np.zeros((CROWS, 512), np.uint8)
            packed[:len(uniq), 0::2] = hi.view(np.uint8)
            packed[:len(uniq), 1::2] = lo.view(np.uint8)
            in_map[f"ctab{h}"] = packed.view(E4)
            for s in range(len(SEGS)):
                seg = invmap[SEG_OFF[s]:SEG_OFF[s] + SEGS[s]]
                in_map[f"idx{s}_{h}"] = _wrap_idx(seg, SEGS[s])

        # boundary rows for tile 0
        bh = np.zeros((4, H), np.float32)
        if t0 % S != 0:
            gtwo = host_gated([t0 - 1, t0 - 2])            # [2, H]
            bh[0] = gtwo[0] * w1 * SG                      # g1[t0-1]
            bh[1] = gtwo[1] * w0 * SG                      # g0[t0-2]
            bh[2] = gtwo[0] * w0 * SG                      # g0[t0-1]
        in_map["bhost"] = bh.astype(E4)

        in_map.update({
            "wkhi": wkhi, "wklo": wklo, "wvhi": wvhi, "wvlo": wvlo,
            "hid": np.ascontiguousarray(hid_bf[t0:t0 + TOK]),
            "w2p1bc": w2p1bc, "w1sbc": w1sbc, "w0sbc": w0sbc,
            "s12": s12, "eb": eb, "consts": consts,
        })
        in_maps.append(in_map)

    nc = _get_program()
    res = run_bass_kernel_spmd(nc, in_maps, list(range(N_CORES)))
    out = np.empty((B * S, H), np.float32)
    for r in range(N_CORES):
        out[r * TOK:(r + 1) * TOK] = res.results[r]["out"].astype(np.float32)
    return out.reshape(B, S, H)


# revision 52
# speedup vs baseline: 1.5538x; 1.0942x over previous
"""EngramModule kernel for Trainium2 (8 NeuronCores, SPMD data-parallel).

v2 architecture (fp8 DoubleRow 3-term GEMMs + transpose-gather):

Per token t (feature dim H=2048):
  idx[t, h]   = hash of n-gram ending at t (8 heads; computed on host)
  memory[t]   = concat_h tables[h, idx[t, h]]
  key_raw     = memory @ Wk.T ; value_raw = memory @ Wv.T
  gate        = sigmoid(dot(hidden, key_raw)/(sqrt(H)*rms_k) - 4)
  g[t]        = gate * value_raw / rms_v           (value_norm folded in conv w)
  out[t]      = g[t]*(1+w2) + w1*g[t-1] + w0*g[t-2]

Device strategy per core (2048 tokens, 16 tiles of 128):
 - Tables are compacted on host per (core, head) to <=2048 unique rows and
   packed as fp8 e4m3 (hi, lo) byte-interleaved 512B rows. One dma_gather
   with transpose=True per (head, segment) delivers memory ALREADY in lhsT
   layout [k-dim on partitions, tokens on free], with (hi, lo) as the two
   bytes of each 16-bit transpose unit -> directly usable as DoubleRow
   operand slots.
 - GEMMs run as 3-term compensated fp8 DoubleRow matmuls (error ~0.1%):
     (Mhi 2-slab)x(Whi 2-slab) + (Mlo)x(Whi) + (Mhi)x(Wlo)
   at 0.25 PE-cycles per output column per 128-contraction.
 - Phase 1 (key): per tile accumulate k in PSUM, ACT-square sumsq + DVE dot
   with bf16 hidden, Newton rsqrt, one ACT Sigmoid -> per-token gate.
 - Phase 2 (value): v in PSUM, sumsq -> rsq_v, g = ACT copy(v * s) bf16;
   g1/g0 cast to fp8; conv = one fp8-DR matmul (shift-pair slots) + one
   fp8-DR boundary matmul per 512-chunk; out = ACT copy(conv psum) + g2.
 - Conv boundary rows for tile 0 are computed EXACTLY on host (2 tokens).
 - Output written bf16, upcast on host.
Only ACT funcs {Square, Sigmoid, Copy} are used -> single act table set,
zero LoadActFuncSet reloads.
"""

import sys

import numpy as np

try:
    import concourse.bass as bass  # noqa: F401
except ImportError:
    sys.path.insert(0, "/opt/trn_rl_repo")

import ml_dtypes

import concourse.bacc as bacc
import concourse.bass as bass
import concourse.tile as tile
from concourse import mybir
from concourse.bass_utils import run_bass_kernel_spmd

E4 = ml_dtypes.float8_e4m3fn
BFNP = ml_dtypes.bfloat16
F32 = mybir.dt.float32
BF16 = mybir.dt.bfloat16
FP8 = mybir.dt.float8e4
I16 = mybir.dt.int16
I32 = mybir.dt.int32
ALU = mybir.AluOpType
AF = mybir.ActivationFunctionType
DR = mybir.MatmulPerfMode.DoubleRow

P = 128
H = 2048
HEADS = 8
HEAD_DIM = 256
VOCAB = 65536
MODULUS = VOCAB - 1
EPS = 1e-6
GATE_BIAS = -4.0
N_CORES = 8
B, S = 4, 4096
TOK = (B * S) // N_CORES        # 2048 tokens per core
NT = TOK // P                   # 16 tiles
CROWS = 2048                    # compact table rows per (core, head)
# gathers batch 4 heads x 256 tokens = 1024 indices (fits the 1024-slot
# SWDGE ring) against the concatenated per-core table [8*2048, 512]
GTOK = 256                      # tokens per gather
NRANGE = TOK // GTOK            # 8 token ranges
NHG = 4                         # head groups of 2
SG = 128.0                      # fp8 scale for g1/g0 (conv operands)
FMAX = 64.0                     # fp8 operand absmax (PSUM partial < ~5.5e4)
NCH = H // 512                  # 4 col chunks of 512


# ---------------------------------------------------------------- host hashing
def _hash_ids_np(ids, mult, off, n):
    """Exact replica of the reference _hash_ids in numpy (wrapping int64)."""
    Bb, Ss = ids.shape
    nh = mult.shape[0]
    ids_u = ids.astype(np.uint64)
    mult_u = mult.astype(np.uint64)
    off_u = off.astype(np.uint64)
    mix = np.zeros((Bb, Ss, nh), dtype=np.uint64)
    for p in range(n):
        shift = n - 1 - p
        tok = np.zeros_like(ids_u)
        if shift > 0:
            tok[:, shift:] = ids_u[:, : Ss - shift]
        else:
            tok = ids_u
        mix ^= tok[:, :, None] * mult_u[None, None, :, p]
    h = (mix + off_u[None, None, :]).view(np.int64)
    hmod = np.remainder(h, MODULUS) + 1
    valid = (np.arange(Ss) >= n - 1)[None, :, None]
    return np.where(valid, hmod, 0)


def _global_indices(input_ids, hm2, ho2, hm3, ho3):
    """[B, S, 8] int32 row indices into the flattened [8*65536, 256] table."""
    h2 = _hash_ids_np(input_ids, hm2, ho2, 2)
    h3 = _hash_ids_np(input_ids, hm3, ho3, 3)
    hid = np.concatenate([h2, h3], axis=-1)          # [B, S, 8]
    gidx = hid + (np.arange(HEADS, dtype=np.int64) * VOCAB)[None, None, :]
    return gidx.astype(np.int32)


def _wrap_idx(inv, n_tok):
    """int16 idx tile [128, n_tok//16]: slot i lives at [i%16, i//16]."""
    t = np.zeros((16, n_tok // 16), np.int16)
    t[np.arange(n_tok) % 16, np.arange(n_tok) // 16] = inv.astype(np.int16)
    return np.ascontiguousarray(np.tile(t, (8, 1)))


# ---------------------------------------------------------------- device program
def build_program():
    nc = bacc.Bacc(None, target_bir_lowering=False)

    t_ctab = nc.dram_tensor("ctab", [HEADS * CROWS, 512], FP8,
                            kind="ExternalInput")
    # idx tiles batched into two tensors: tiny range-0 block loads first so
    # the first gathers start immediately. 64 int16 words per gather.
    IDXW0 = (HEADS // NHG * GTOK) // 16 * NHG
    IDXWR = (HEADS // NHG * GTOK) // 16 * NHG * (NRANGE - 1)
    t_idx0 = nc.dram_tensor("idx0", [P, IDXW0], I16, kind="ExternalInput")
    t_idxr = nc.dram_tensor("idxr", [P, IDXWR], I16, kind="ExternalInput")
    t_wkhi = nc.dram_tensor("wkhi", [P, HEADS, 2, H], FP8, kind="ExternalInput")
    t_wklo = nc.dram_tensor("wklo", [P, HEADS, 2, H], FP8, kind="ExternalInput")
    t_wvhi = nc.dram_tensor("wvhi", [P, HEADS, 2, H], FP8, kind="ExternalInput")
    t_wvlo = nc.dram_tensor("wvlo", [P, HEADS, 2, H], FP8, kind="ExternalInput")
    t_hid = nc.dram_tensor("hid", [TOK, H], BF16, kind="ExternalInput")
    t_w2p1 = nc.dram_tensor("w2p1bc", [P, H], BF16, kind="ExternalInput")
    t_w1s = nc.dram_tensor("w1sbc", [P, H], BF16, kind="ExternalInput")
    t_w0s = nc.dram_tensor("w0sbc", [P, H], BF16, kind="ExternalInput")
    t_s12 = nc.dram_tensor("s12", [P, 2, P], FP8, kind="ExternalInput")
    t_eb = nc.dram_tensor("eb", [P, 2, P], FP8, kind="ExternalInput")
    t_bh = nc.dram_tensor("bhost", [4, H], FP8, kind="ExternalInput")
    t_out = nc.dram_tensor("out", [TOK, H], BF16, kind="ExternalOutput")

    # scalar constants (host-computed, folded scales)
    t_consts = nc.dram_tensor("consts", [P, 4], F32, kind="ExternalInput")
    # consts columns: 0 = eps_k' = (sM*sWk)^2 * H * EPS
    #                 1 = eps_v' = (sM*sWv)^2 * H * EPS
    #                 2 = sqrt(H)
    #                 3 = GATE_BIAS

    with tile.TileContext(nc) as tc:
        with (
            tc.tile_pool(name="pconst", bufs=1) as pc,
            tc.tile_pool(name="pM", bufs=1) as pM,
            tc.tile_pool(name="pWvpre", bufs=1) as pWvpre,
            tc.tile_pool(name="pstat", bufs=4) as pst,
            tc.tile_pool(name="pscr", bufs=2) as pscr,
            tc.tile_pool(name="pbig", bufs=3, space="PSUM") as pbig,
        ):
            # ---- small consts
            idx0_sb = pc.tile([P, IDXW0], I16)
            nc.sync.dma_start(out=idx0_sb[:], in_=t_idx0[:])
            consts = pc.tile([P, 4], F32)
            nc.sync.dma_start(out=consts[:], in_=t_consts[:])
            idxr_sb = pc.tile([P, IDXWR], I16)
            nc.sync.dma_start(out=idxr_sb[:], in_=t_idxr[:])
            s12_sb = pc.tile([P, 2, P], FP8)
            eb_sb = pc.tile([P, 2, P], FP8)
            sg_all = pc.tile([P, NT], F32)      # per-tile gate scalars

            # ---- gathers: (token-range, head-group) rectangles of 1024 idx
            HPG = HEADS // NHG  # heads per gather
            NIG = HPG * GTOK    # idxs per gather
            mseg = [[None] * NHG for _ in range(NRANGE)]
            for r in range(NRANGE):
                for hg in range(NHG):
                    g = r * NHG + hg
                    W = NIG // 16
                    if r == 0:
                        iap = idx0_sb[:, hg * W:(hg + 1) * W]
                    else:
                        w0 = (g - NHG) * W
                        iap = idxr_sb[:, w0:w0 + W]
                    m = pM.tile([P, 4 * NIG], FP8, name=f"m{r}_{hg}")
                    nc.gpsimd.dma_gather(
                        out_ap=m[:].rearrange("p (f t) -> p f t", f=4),
                        in_ap=t_ctab[:],
                        idxs_ap=iap,
                        num_idxs=NIG, num_idxs_reg=NIG,
                        elem_size=512, transpose=True)
                    mseg[r][hg] = m

            def m_slabs(h, i):
                """(lhsT_hi, lhsT_lo) [p, c(2), t(128)] for tile i, head h."""
                m = mseg[i // 2][h // (HEADS // NHG)]
                t0 = (h % (HEADS // NHG)) * GTOK + (i % 2) * P
                ctj = m[:].rearrange("p (c t j) -> p c t j", c=2, t=NIG, j=2)
                return (ctj[:, :, t0:t0 + P, 0], ctj[:, :, t0:t0 + P, 1])

            def gemm_terms(i, whi_sb, wlo_sb, ph, terms, start, stop):
                """Emit a subset of the 3-term fp8 DR GEMM for tile i into
                psum halves ph[0|1]. Terms: 0 = Mhi@Whi, 1 = Mlo@Whi,
                2 = Mhi@Wlo. start/stop apply at the first/last emitted
                matmul of each psum chunk group."""
                for tx, term in enumerate(terms):
                    for h in range(HEADS):
                        hi, lo = m_slabs(h, i)
                        mop = hi if term != 1 else lo
                        wsb = whi_sb if term != 2 else wlo_sb
                        for ch in range(NCH):
                            pt = ph[ch // 2]
                            cs = slice((ch % 2) * 512, (ch % 2) * 512 + 512)
                            nc.tensor.matmul(
                                pt[:, cs], lhsT=mop,
                                rhs=wsb[:, h, :, ch * 512:(ch + 1) * 512],
                                start=(start and tx == 0 and h == 0),
                                stop=(stop and tx == len(terms) - 1
                                      and h == HEADS - 1),
                                perf_mode=DR)

            def newton_rsqrt(u):
                """In-place u <- 1/sqrt(u) via bit-seed + 2 Newton iters."""
                y = pst.tile([P, 1], F32, tag="ny")
                yi = y[:].bitcast(I32)
                nc.vector.tensor_scalar(out=yi, in0=u[:].bitcast(I32), scalar1=1,
                                        scalar2=None, op0=ALU.logical_shift_right)
                nc.vector.tensor_scalar(out=yi, in0=yi, scalar1=-1,
                                        scalar2=0x5F3759DF, op0=ALU.mult,
                                        op1=ALU.add)
                t2 = pst.tile([P, 1], F32, tag="nt")
                for _ in range(2):
                    nc.vector.tensor_mul(out=t2[:], in0=y[:], in1=y[:])
                    nc.vector.tensor_mul(out=t2[:], in0=t2[:], in1=u[:])
                    nc.vector.tensor_scalar(out=t2[:], in0=t2[:], scalar1=-0.5,
                                            scalar2=1.5, op0=ALU.mult, op1=ALU.add)
                    nc.vector.tensor_mul(out=y[:], in0=y[:], in1=t2[:])
                return y

            # ================= PHASE 1: key =================
            wvlo_sb = pWvpre.tile([P, HEADS, 2, H], FP8)

            def key_epi(i, ph, hid_sb):
                # sumsq + dot per half
                sq2 = pst.tile([P, 2], F32, tag="sq2")
                dt2 = pst.tile([P, 2], F32, tag="dt2")
                for x in range(2):
                    scr = pscr.tile([P, 1024], F32, tag="scr")
                    nc.scalar.activation(out=scr[:], in_=ph[x][:],
                                         func=AF.Square,
                                         accum_out=sq2[:, x:x + 1])
                    scr = pscr.tile([P, 1024], F32, tag="scr")
                    nc.vector.scalar_tensor_tensor(
                        out=scr[:], in0=ph[x][:], scalar=1.0,
                        in1=hid_sb[:, x * 1024:(x + 1) * 1024],
                        op0=ALU.mult, op1=ALU.mult,
                        accum_out=dt2[:, x:x + 1])
                u = pst.tile([P, 1], F32, tag="u")
                nc.vector.tensor_reduce(out=u[:], in_=sq2[:],
                                        axis=mybir.AxisListType.X, op=ALU.add)
                # u = sumsq + eps_k'  (rsq' = 1/sqrt(u) folds the /sqrt(H))
                nc.vector.tensor_scalar(out=u[:], in0=u[:],
                                        scalar1=consts[:, 0:1], scalar2=None,
                                        op0=ALU.add)
                rsq = newton_rsqrt(u)
                dot = pst.tile([P, 1], F32, tag="dot")
                nc.vector.tensor_reduce(out=dot[:], in_=dt2[:],
                                        axis=mybir.AxisListType.X, op=ALU.add)
                # gate = sigmoid(dot * rsq' - 4)
                nc.scalar.activation(out=sg_all[:, i:i + 1], in_=dot[:],
                                     func=AF.Sigmoid, scale=rsq[:],
                                     bias=consts[:, 3:4])

            with (
                tc.tile_pool(name="pWk", bufs=1) as pWk,
                tc.tile_pool(name="phid", bufs=3) as phid,
                tc.tile_pool(name="pkx", bufs=1, space="PSUM") as pkx,
            ):
                wkhi_sb = pWk.tile([P, HEADS, 2, H], FP8)
                wklo_sb = pWk.tile([P, HEADS, 2, H], FP8)
                for h in range(HEADS):
                    nc.sync.dma_start(out=wkhi_sb[:, h], in_=t_wkhi[:, h])
                for h in range(HEADS):
                    nc.sync.dma_start(out=wklo_sb[:, h], in_=t_wklo[:, h])

                # tiles 0 and 1 are software-pipelined term-wise: their Whi
                # terms run while Wk_lo is still streaming in
                hid01 = []
                ph01 = []
                for i in range(2):
                    hid_sb = phid.tile([P, H], BF16, tag="hid")
                    nc.sync.dma_start(out=hid_sb[:],
                                      in_=t_hid[i * P:(i + 1) * P, :])
                    hid01.append(hid_sb)
                ph01.append([pbig.tile([P, 1024], F32, tag="ps", name="k0_0"),
                             pbig.tile([P, 1024], F32, tag="ps", name="k0_1")])
                ph01.append([pbig.tile([P, 1024], F32, tag="ps", name="k1_0"),
                             pkx.tile([P, 1024], F32, name="k1_1")])
                gemm_terms(0, wkhi_sb, wklo_sb, ph01[0], (0, 1), True, False)
                gemm_terms(1, wkhi_sb, wklo_sb, ph01[1], (0, 1), True, False)
                gemm_terms(0, wkhi_sb, wklo_sb, ph01[0], (2,), False, True)
                key_epi(0, ph01[0], hid01[0])
                gemm_terms(1, wkhi_sb, wklo_sb, ph01[1], (2,), False, True)
                key_epi(1, ph01[1], hid01[1])

                for i in range(2, NT):
                    if i == 8:
                        # mid-key prefetch of Wv_lo (Pool queue; DMA engines
                        # are free of startup traffic by now)
                        for h in range(HEADS):
                            nc.gpsimd.dma_start(out=wvlo_sb[:, h],
                                                in_=t_wvlo[:, h])
                    hid_sb = phid.tile([P, H], BF16, tag="hid")
                    nc.sync.dma_start(out=hid_sb[:],
                                      in_=t_hid[i * P:(i + 1) * P, :])
                    ph = [pbig.tile([P, 1024], F32, tag="ps", name=f"k{i}_{x}")
                          for x in range(2)]
                    gemm_terms(i, wkhi_sb, wklo_sb, ph, (0, 1, 2), True, True)
                    key_epi(i, ph, hid_sb)

            # ================= PHASE 2: value =================
            with (
                tc.tile_pool(name="pWvhi", bufs=1) as pWvhi,
                tc.tile_pool(name="pg", bufs=2) as pg,
                tc.tile_pool(name="pout", bufs=2) as pout,
                tc.tile_pool(name="pb", bufs=1) as pb,
                tc.tile_pool(name="pconv", bufs=2, space="PSUM") as pconv,
            ):
                # Wv_hi streamed on SP first; the small constant tiles after
                # it so they don't steal DMA-engine slots from the stream
                wvhi_sb = pWvhi.tile([P, HEADS, 2, H], FP8)
                for h in range(HEADS):
                    nc.sync.dma_start(out=wvhi_sb[:, h], in_=t_wvhi[:, h])
                w2p1_sb = pWvhi.tile([P, H], BF16)
                w1s_sb = pWvhi.tile([P, H], BF16)
                w0s_sb = pWvhi.tile([P, H], BF16)
                nc.sync.dma_start(out=w2p1_sb[:], in_=t_w2p1[:])
                nc.sync.dma_start(out=w1s_sb[:], in_=t_w1s[:])
                nc.sync.dma_start(out=w0s_sb[:], in_=t_w0s[:])
                nc.sync.dma_start(out=s12_sb[:], in_=t_s12[:])
                nc.sync.dma_start(out=eb_sb[:], in_=t_eb[:])

                # boundary ping-pong tiles (fixed, fully memset once so reads
                # of untouched rows are well-defined); tile 0's rows from host
                b_tiles = [pb.tile([P, 2, H], FP8, name=f"b{x}")
                           for x in range(2)]
                nc.vector.memset(b_tiles[0][:], 0.0)
                nc.vector.memset(b_tiles[1][:], 0.0)
                nc.sync.dma_start(out=b_tiles[0][0:4, 0, :], in_=t_bh[:])

                for i in range(NT):
                    b_prev = b_tiles[i % 2]
                    ph = [pbig.tile([P, 1024], F32, tag="ps", name=f"v{i}_{x}")
                          for x in range(2)]
                    gemm_terms(i, wvhi_sb, wvlo_sb, ph, (2, 0, 1), True, True)
                    sq2 = pst.tile([P, 2], F32, tag="sq2")
                    scr = pscr.tile([P, 1024], F32, tag="scr")
                    for x in range(2):
                        nc.scalar.activation(out=scr[:], in_=ph[x][:],
                                             func=AF.Square,
                                             accum_out=sq2[:, x:x + 1])
                    u = pst.tile([P, 1], F32, tag="u")
                    nc.vector.tensor_reduce(out=u[:], in_=sq2[:],
                                            axis=mybir.AxisListType.X,
                                            op=ALU.add)
                    nc.vector.tensor_scalar(out=u[:], in0=u[:],
                                            scalar1=consts[:, 1:2], scalar2=None,
                                            op0=ALU.add)
                    rsq = newton_rsqrt(u)
                    # s_final = rsq * sqrt(H) * gate
                    sfin = pst.tile([P, 1], F32, tag="sfin")
                    nc.vector.scalar_tensor_tensor(
                        out=sfin[:], in0=rsq[:], scalar=consts[:, 2:3],
                        in1=sg_all[:, i:i + 1], op0=ALU.mult, op1=ALU.mult)
                    # g = v * s_final   (bf16)
                    g = pg.tile([P, H], BF16, tag="g")
                    for x in range(2):
                        nc.scalar.activation(
                            out=g[:, x * 1024:(x + 1) * 1024], in_=ph[x][:],
                            func=AF.Copy, scale=sfin[:])
                    # g2 bf16; g1/g0 -> fp8 slots of g10
                    g2 = pg.tile([P, H], BF16, tag="g2")
                    nc.vector.tensor_mul(out=g2[:], in0=g[:], in1=w2p1_sb[:])
                    g10 = pg.tile([P, 2, H], FP8, tag="g10")
                    nc.vector.tensor_mul(out=g10[:, 0, :], in0=g[:], in1=w1s_sb[:])
                    nc.vector.tensor_mul(out=g10[:, 1, :], in0=g[:], in1=w0s_sb[:])

                    # conv psum per 512-chunk: shifts (DR) + boundary (DR)
                    out_sb = pout.tile([P, H], BF16, tag="out")
                    for ch in range(NCH):
                        cs = slice(ch * 512, (ch + 1) * 512)
                        pcv = pconv.tile([P, 512], F32, tag="pcv")
                        nc.tensor.matmul(pcv[:], lhsT=s12_sb[:],
                                         rhs=g10[:, :, cs],
                                         start=True, stop=False, perf_mode=DR)
                        nc.tensor.matmul(pcv[:], lhsT=eb_sb[:],
                                         rhs=b_prev[:, :, cs],
                                         start=False, stop=True, perf_mode=DR)
                        nc.scalar.activation(out=out_sb[:, cs], in_=pcv[:],
                                             func=AF.Copy)
                    nc.vector.tensor_add(out=out_sb[:], in0=out_sb[:], in1=g2[:])
                    nc.sync.dma_start(out=t_out[i * P:(i + 1) * P, :],
                                      in_=out_sb[:])

                    # boundary rows for next tile from g10 tails
                    if i < NT - 1:
                        b_next = b_tiles[(i + 1) % 2]
                        nc.sync.dma_start(out=b_next[0:1, 0, :],
                                          in_=g10[127:128, 0, :])
                        nc.sync.dma_start(out=b_next[1:3, 0, :],
                                          in_=g10[126:128, 1, :])

    nc.compile()
    return nc


# ---------------------------------------------------------------- host wrapper
_PROGRAM = None


def _get_program():
    global _PROGRAM
    if _PROGRAM is None:
        _PROGRAM = build_program()
    return _PROGRAM


def kernel(hidden_states, input_ids, tables, Wk, Wv, key_norm_w, value_norm_w,
           conv_w, hm2, ho2, hm3, ho3):
    hidden_states = np.asarray(hidden_states, dtype=np.float32)
    input_ids = np.asarray(input_ids, dtype=np.int64)
    tables = np.asarray(tables, dtype=np.float32)
    Wk = np.asarray(Wk, dtype=np.float32)
    Wv = np.asarray(Wv, dtype=np.float32)
    key_norm_w = np.asarray(key_norm_w, dtype=np.float32)
    value_norm_w = np.asarray(value_norm_w, dtype=np.float32)
    conv_w = np.asarray(conv_w, dtype=np.float32)

    gidx = _global_indices(input_ids, np.asarray(hm2), np.asarray(ho2),
                           np.asarray(hm3), np.asarray(ho3))   # [B,S,8] i32
    hid_local = (gidx % VOCAB).reshape(B * S, HEADS)           # per-head rows

    # fold key_norm into hidden; fp8 scales
    hid_flat = (hidden_states.reshape(B * S, H) * key_norm_w[None, :])
    hid_bf = hid_flat.astype(BFNP)

    sM = FMAX / max(np.abs(tables).max(), 1e-30)
    sWk = FMAX / max(np.abs(Wk).max(), 1e-30)
    sWv = FMAX / max(np.abs(Wv).max(), 1e-30)

    def split_fp8(x, s):
        hi = (x * s).astype(E4)
        lo = ((x * s) - hi.astype(np.float32)).astype(E4)
        return hi, lo

    def w_layout(W, s):
        # [p, head, c, n] with W.T[k, n] = W[n, k]; k = h*256 + c*128 + p
        hi, lo = split_fp8(np.ascontiguousarray(W.T), s)   # [k, n]
        def lay(a):
            return np.ascontiguousarray(
                a.reshape(HEADS, 2, P, H).transpose(2, 0, 1, 3))
        return lay(hi), lay(lo)

    wkhi, wklo = w_layout(Wk, sWk)
    wvhi, wvlo = w_layout(Wv, sWv)

    # conv weight foldings (value_norm + fp8 g-scale)
    w0 = conv_w[:, 0] * value_norm_w
    w1 = conv_w[:, 1] * value_norm_w
    w2p1 = (1.0 + conv_w[:, 2]) * value_norm_w
    w2p1bc = np.ascontiguousarray(np.broadcast_to(w2p1, (P, H))).astype(BFNP)
    w1sbc = np.ascontiguousarray(np.broadcast_to(w1 * SG, (P, H))).astype(BFNP)
    w0sbc = np.ascontiguousarray(np.broadcast_to(w0 * SG, (P, H))).astype(BFNP)

    # shift-pair (s1, s2) and boundary lhsT matrices, scaled by 1/SG
    inv = np.float32(1.0 / SG)
    s12 = np.zeros((P, 2, P), E4)
    s12[:, 0, :] = (np.eye(P, k=1, dtype=np.float32) * inv).astype(E4)
    s12[:, 1, :] = (np.eye(P, k=2, dtype=np.float32) * inv).astype(E4)
    eb = np.zeros((P, 2, P), E4)
    # b_pad rows: 0 -> out0 (g1[127]), 1 -> out0 (g0[126]), 2 -> out1 (g0[127])
    eb[0, 0, 0] = E4(inv)
    eb[1, 0, 0] = E4(inv)
    eb[2, 0, 1] = E4(inv)

    consts = np.zeros((P, 4), np.float32)
    consts[:, 0] = (sM * sWk) ** 2 * H * EPS
    consts[:, 1] = (sM * sWv) ** 2 * H * EPS
    consts[:, 2] = np.sqrt(np.float32(H))
    consts[:, 3] = GATE_BIAS

    # exact host reference for the 2 boundary tokens of each core
    def host_gated(trange):
        """gated[t] rows (f64->f32) for global token indices trange."""
        out = np.zeros((len(trange), H), np.float32)
        tabs = tables.astype(np.float64)
        for j, t in enumerate(trange):
            rows = [tabs[h, hid_local[t, h]] for h in range(HEADS)]
            mem = np.concatenate(rows)                     # [2048]
            kr = mem @ Wk.T.astype(np.float64)
            vr = mem @ Wv.T.astype(np.float64)
            rk = 1.0 / np.sqrt(np.mean(kr ** 2) + EPS)
            rv = 1.0 / np.sqrt(np.mean(vr ** 2) + EPS)
            z = float(hid_flat[t].astype(np.float64) @ (kr * rk)) / np.sqrt(H) \
                + GATE_BIAS
            gate = 1.0 / (1.0 + np.exp(-z))
            out[j] = (gate * (vr * rv) * value_norm_w).astype(np.float32)
        return out

    in_maps = []
    for r in range(N_CORES):
        t0 = r * TOK
        idx_core = hid_local[t0:t0 + TOK]                  # [2048, 8]
        in_map = {}
        packed = np.zeros((HEADS * CROWS, 512), np.uint8)
        gidx16 = np.empty((TOK, HEADS), np.int16)          # h*2048 + inv
        for h in range(HEADS):
            uniq, invmap = np.unique(idx_core[:, h], return_inverse=True)
            rows = tables[h, uniq]                         # [n_u, 256]
            hi, lo = split_fp8(rows, sM)
            blk = packed[h * CROWS:h * CROWS + len(uniq)]
            blk[:, 0::2] = hi.view(np.uint8)
            blk[:, 1::2] = lo.view(np.uint8)
            gidx16[:, h] = (h * CROWS + invmap).astype(np.int16)
        in_map["ctab"] = packed.view(E4)
        # gather idx rectangles: (token-range r, head-group hg): the 1024
        # idxs are head-major (4 heads x 256 tokens) to match m_slabs
        idx_cols = []
        for r_ in range(NRANGE):
            for hg in range(NHG):
                hpg = HEADS // NHG
                rect = gidx16[r_ * GTOK:(r_ + 1) * GTOK,
                              hg * hpg:(hg + 1) * hpg]
                seq = np.ascontiguousarray(rect.T).reshape(-1)   # head-major
                idx_cols.append(_wrap_idx(seq, hpg * GTOK))
        in_map["idx0"] = np.ascontiguousarray(
            np.concatenate(idx_cols[:NHG], axis=1))
        in_map["idxr"] = np.ascontiguousarray(
            np.concatenate(idx_cols[NHG:], axis=1))

        # boundary rows for tile 0
        bh = np.zeros((4, H), np.float32)
        if t0 % S != 0:
            gtwo = host_gated([t0 - 1, t0 - 2])            # [2, H]
            bh[0] = gtwo[0] * w1 * SG                      # g1[t0-1]
            bh[1] = gtwo[1] * w0 * SG                      # g0[t0-2]
            bh[2] = gtwo[0] * w0 * SG                      # g0[t0-1]
        in_map["bhost"] = bh.astype(E4)

        in_map.update({
            "wkhi": wkhi, "wklo": wklo, "wvhi": wvhi, "wvlo": wvlo,
            "hid": np.ascontiguousarray(hid_bf[t0:t0 + TOK]),
            "w2p1bc": w2p1bc, "w1sbc": w1sbc, "w0sbc": w0sbc,
            "s12": s12, "eb": eb, "consts": consts,
        })
        in_maps.append(in_map)

    nc = _get_program()
    res = run_bass_kernel_spmd(nc, in_maps, list(range(N_CORES)))
    out = np.empty((B * S, H), np.float32)
    for r in range(N_CORES):
        out[r * TOK:(r + 1) * TOK] = res.results[r]["out"].astype(np.float32)
    return out.reshape(B, S, H)


# revision 54
# speedup vs baseline: 1.5847x; 1.0198x over previous
"""EngramModule kernel for Trainium2 (8 NeuronCores, SPMD data-parallel).

v2 architecture (fp8 DoubleRow 3-term GEMMs + transpose-gather):

Per token t (feature dim H=2048):
  idx[t, h]   = hash of n-gram ending at t (8 heads; computed on host)
  memory[t]   = concat_h tables[h, idx[t, h]]
  key_raw     = memory @ Wk.T ; value_raw = memory @ Wv.T
  gate        = sigmoid(dot(hidden, key_raw)/(sqrt(H)*rms_k) - 4)
  g[t]        = gate * value_raw / rms_v           (value_norm folded in conv w)
  out[t]      = g[t]*(1+w2) + w1*g[t-1] + w0*g[t-2]

Device strategy per core (2048 tokens, 16 tiles of 128):
 - Tables are compacted on host per (core, head) to <=2048 unique rows and
   packed as fp8 e4m3 (hi, lo) byte-interleaved 512B rows. One dma_gather
   with transpose=True per (head, segment) delivers memory ALREADY in lhsT
   layout [k-dim on partitions, tokens on free], with (hi, lo) as the two
   bytes of each 16-bit transpose unit -> directly usable as DoubleRow
   operand slots.
 - GEMMs run as 3-term compensated fp8 DoubleRow matmuls (error ~0.1%):
     (Mhi 2-slab)x(Whi 2-slab) + (Mlo)x(Whi) + (Mhi)x(Wlo)
   at 0.25 PE-cycles per output column per 128-contraction.
 - Phase 1 (key): per tile accumulate k in PSUM, ACT-square sumsq + DVE dot
   with bf16 hidden, Newton rsqrt, one ACT Sigmoid -> per-token gate.
 - Phase 2 (value): v in PSUM, sumsq -> rsq_v, g = ACT copy(v * s) bf16;
   g1/g0 cast to fp8; conv = one fp8-DR matmul (shift-pair slots) + one
   fp8-DR boundary matmul per 512-chunk; out = ACT copy(conv psum) + g2.
 - Conv boundary rows for tile 0 are computed EXACTLY on host (2 tokens).
 - Output written bf16, upcast on host.
Only ACT funcs {Square, Sigmoid, Copy} are used -> single act table set,
zero LoadActFuncSet reloads.
"""

import sys

import numpy as np

try:
    import concourse.bass as bass  # noqa: F401
except ImportError:
    sys.path.insert(0, "/opt/trn_rl_repo")

import ml_dtypes

import concourse.bacc as bacc
import concourse.bass as bass
import concourse.tile as tile
from concourse import mybir
from concourse.bass_utils import run_bass_kernel_spmd

E4 = ml_dtypes.float8_e4m3fn
BFNP = ml_dtypes.bfloat16
F32 = mybir.dt.float32
BF16 = mybir.dt.bfloat16
FP8 = mybir.dt.float8e4
I16 = mybir.dt.int16
I32 = mybir.dt.int32
ALU = mybir.AluOpType
AF = mybir.ActivationFunctionType
DR = mybir.MatmulPerfMode.DoubleRow

P = 128
H = 2048
HEADS = 8
HEAD_DIM = 256
VOCAB = 65536
MODULUS = VOCAB - 1
EPS = 1e-6
GATE_BIAS = -4.0
N_CORES = 8
B, S = 4, 4096
TOK = (B * S) // N_CORES        # 2048 tokens per core
NT = TOK // P                   # 16 tiles
CROWS = 2048                    # compact table rows per (core, head)
# gathers batch 4 heads x 256 tokens = 1024 indices (fits the 1024-slot
# SWDGE ring) against the concatenated per-core table [8*2048, 512]
GTOK = 256                      # tokens per gather
NRANGE = TOK // GTOK            # 8 token ranges
NHG = 4                         # head groups of 2
SG = 128.0                      # fp8 scale for g1/g0 (conv operands)
FMAX = 64.0                     # fp8 operand absmax (PSUM partial < ~5.5e4)
NCH = H // 512                  # 4 col chunks of 512


# ---------------------------------------------------------------- host hashing
def _hash_ids_np(ids, mult, off, n):
    """Exact replica of the reference _hash_ids in numpy (wrapping int64)."""
    Bb, Ss = ids.shape
    nh = mult.shape[0]
    ids_u = ids.astype(np.uint64)
    mult_u = mult.astype(np.uint64)
    off_u = off.astype(np.uint64)
    mix = np.zeros((Bb, Ss, nh), dtype=np.uint64)
    for p in range(n):
        shift = n - 1 - p
        tok = np.zeros_like(ids_u)
        if shift > 0:
            tok[:, shift:] = ids_u[:, : Ss - shift]
        else:
            tok = ids_u
        mix ^= tok[:, :, None] * mult_u[None, None, :, p]
    h = (mix + off_u[None, None, :]).view(np.int64)
    hmod = np.remainder(h, MODULUS) + 1
    valid = (np.arange(Ss) >= n - 1)[None, :, None]
    return np.where(valid, hmod, 0)


def _global_indices(input_ids, hm2, ho2, hm3, ho3):
    """[B, S, 8] int32 row indices into the flattened [8*65536, 256] table."""
    h2 = _hash_ids_np(input_ids, hm2, ho2, 2)
    h3 = _hash_ids_np(input_ids, hm3, ho3, 3)
    hid = np.concatenate([h2, h3], axis=-1)          # [B, S, 8]
    gidx = hid + (np.arange(HEADS, dtype=np.int64) * VOCAB)[None, None, :]
    return gidx.astype(np.int32)


def _wrap_idx(inv, n_tok):
    """int16 idx tile [128, n_tok//16]: slot i lives at [i%16, i//16]."""
    t = np.zeros((16, n_tok // 16), np.int16)
    t[np.arange(n_tok) % 16, np.arange(n_tok) // 16] = inv.astype(np.int16)
    return np.ascontiguousarray(np.tile(t, (8, 1)))


# ---------------------------------------------------------------- device program
def build_program():
    nc = bacc.Bacc(None, target_bir_lowering=False)

    t_ctab = nc.dram_tensor("ctab", [HEADS * CROWS, 512], FP8,
                            kind="ExternalInput")
    # idx tiles batched into two tensors: tiny range-0 block loads first so
    # the first gathers start immediately. 64 int16 words per gather.
    IDXW0 = (HEADS // NHG * GTOK) // 16 * NHG
    IDXWR = (HEADS // NHG * GTOK) // 16 * NHG * (NRANGE - 1)
    t_idx0 = nc.dram_tensor("idx0", [P, IDXW0], I16, kind="ExternalInput")
    t_idxr = nc.dram_tensor("idxr", [P, IDXWR], I16, kind="ExternalInput")
    t_wkhi = nc.dram_tensor("wkhi", [P, HEADS, 2, H], FP8, kind="ExternalInput")
    t_wklo = nc.dram_tensor("wklo", [P, HEADS, 2, H], FP8, kind="ExternalInput")
    t_wvhi = nc.dram_tensor("wvhi", [P, HEADS, 2, H], FP8, kind="ExternalInput")
    t_wvlo = nc.dram_tensor("wvlo", [P, HEADS, 2, H], FP8, kind="ExternalInput")
    t_hid = nc.dram_tensor("hid", [TOK, H], BF16, kind="ExternalInput")
    t_w2p1 = nc.dram_tensor("w2p1bc", [P, H], BF16, kind="ExternalInput")
    t_w1s = nc.dram_tensor("w1sbc", [P, H], BF16, kind="ExternalInput")
    t_w0s = nc.dram_tensor("w0sbc", [P, H], BF16, kind="ExternalInput")
    t_s12 = nc.dram_tensor("s12", [P, 2, P], FP8, kind="ExternalInput")
    t_eb = nc.dram_tensor("eb", [P, 2, P], FP8, kind="ExternalInput")
    t_bh = nc.dram_tensor("bhost", [4, H], FP8, kind="ExternalInput")
    t_out = nc.dram_tensor("out", [TOK, H], BF16, kind="ExternalOutput")

    # scalar constants (host-computed, folded scales)
    t_consts = nc.dram_tensor("consts", [P, 4], F32, kind="ExternalInput")
    # consts columns: 0 = eps_k' = (sM*sWk)^2 * H * EPS
    #                 1 = eps_v' = (sM*sWv)^2 * H * EPS
    #                 2 = sqrt(H)
    #                 3 = GATE_BIAS

    with tile.TileContext(nc) as tc:
        with (
            tc.tile_pool(name="pconst", bufs=1) as pc,
            tc.tile_pool(name="pM", bufs=1) as pM,
            tc.tile_pool(name="pWvpre", bufs=1) as pWvpre,
            tc.tile_pool(name="pstat", bufs=4) as pst,
            tc.tile_pool(name="pscr", bufs=2) as pscr,
            tc.tile_pool(name="pbig", bufs=3, space="PSUM") as pbig,
        ):
            # ---- small consts
            idx0_sb = pc.tile([P, IDXW0], I16)
            nc.sync.dma_start(out=idx0_sb[:], in_=t_idx0[:])
            consts = pc.tile([P, 4], F32)
            nc.sync.dma_start(out=consts[:], in_=t_consts[:])
            idxr_sb = pc.tile([P, IDXWR], I16)
            nc.sync.dma_start(out=idxr_sb[:], in_=t_idxr[:])
            s12_sb = pc.tile([P, 2, P], FP8)
            eb_sb = pc.tile([P, 2, P], FP8)
            sg_all = pc.tile([P, NT], F32)      # per-tile gate scalars

            # ---- gathers: (token-range, head-group) rectangles of 1024 idx
            HPG = HEADS // NHG  # heads per gather
            NIG = HPG * GTOK    # idxs per gather
            mseg = [[None] * NHG for _ in range(NRANGE)]

            def emit_gathers(ranges):
                for r in ranges:
                    for hg in range(NHG):
                        g = r * NHG + hg
                        W = NIG // 16
                        if r == 0:
                            iap = idx0_sb[:, hg * W:(hg + 1) * W]
                        else:
                            w0 = (g - NHG) * W
                            iap = idxr_sb[:, w0:w0 + W]
                        m = pM.tile([P, 4 * NIG], FP8, name=f"m{r}_{hg}")
                        nc.gpsimd.dma_gather(
                            out_ap=m[:].rearrange("p (f t) -> p f t", f=4),
                            in_ap=t_ctab[:],
                            idxs_ap=iap,
                            num_idxs=NIG, num_idxs_reg=NIG,
                            elem_size=512, transpose=True)
                        mseg[r][hg] = m

            emit_gathers(range(2))

            def m_slabs(h, i):
                """(lhsT_hi, lhsT_lo) [p, c(2), t(128)] for tile i, head h."""
                m = mseg[i // 2][h // (HEADS // NHG)]
                t0 = (h % (HEADS // NHG)) * GTOK + (i % 2) * P
                ctj = m[:].rearrange("p (c t j) -> p c t j", c=2, t=NIG, j=2)
                return (ctj[:, :, t0:t0 + P, 0], ctj[:, :, t0:t0 + P, 1])

            def gemm_terms(i, whi_sb, wlo_sb, ph, terms, start, stop):
                """Emit a subset of the 3-term fp8 DR GEMM for tile i into
                psum halves ph[0|1]. Terms: 0 = Mhi@Whi, 1 = Mlo@Whi,
                2 = Mhi@Wlo. start/stop apply at the first/last emitted
                matmul of each psum chunk group."""
                for tx, term in enumerate(terms):
                    for h in range(HEADS):
                        hi, lo = m_slabs(h, i)
                        mop = hi if term != 1 else lo
                        wsb = whi_sb if term != 2 else wlo_sb
                        for ch in range(NCH):
                            pt = ph[ch // 2]
                            cs = slice((ch % 2) * 512, (ch % 2) * 512 + 512)
                            nc.tensor.matmul(
                                pt[:, cs], lhsT=mop,
                                rhs=wsb[:, h, :, ch * 512:(ch + 1) * 512],
                                start=(start and tx == 0 and h == 0),
                                stop=(stop and tx == len(terms) - 1
                                      and h == HEADS - 1),
                                perf_mode=DR)

            def newton_rsqrt(u):
                """In-place u <- 1/sqrt(u) via bit-seed + 2 Newton iters."""
                y = pst.tile([P, 1], F32, tag="ny")
                yi = y[:].bitcast(I32)
                nc.vector.tensor_scalar(out=yi, in0=u[:].bitcast(I32), scalar1=1,
                                        scalar2=None, op0=ALU.logical_shift_right)
                nc.vector.tensor_scalar(out=yi, in0=yi, scalar1=-1,
                                        scalar2=0x5F3759DF, op0=ALU.mult,
                                        op1=ALU.add)
                t2 = pst.tile([P, 1], F32, tag="nt")
                for _ in range(2):
                    nc.vector.tensor_mul(out=t2[:], in0=y[:], in1=y[:])
                    nc.vector.tensor_mul(out=t2[:], in0=t2[:], in1=u[:])
                    nc.vector.tensor_scalar(out=t2[:], in0=t2[:], scalar1=-0.5,
                                            scalar2=1.5, op0=ALU.mult, op1=ALU.add)
                    nc.vector.tensor_mul(out=y[:], in0=y[:], in1=t2[:])
                return y

            # ================= PHASE 1: key =================
            wvlo_sb = pWvpre.tile([P, HEADS, 2, H], FP8)

            def key_epi(i, ph, hid_sb):
                # sumsq + dot per half
                sq2 = pst.tile([P, 2], F32, tag="sq2")
                dt2 = pst.tile([P, 2], F32, tag="dt2")
                for x in range(2):
                    scr = pscr.tile([P, 1024], F32, tag="scr")
                    nc.scalar.activation(out=scr[:], in_=ph[x][:],
                                         func=AF.Square,
                                         accum_out=sq2[:, x:x + 1])
                    scr = pscr.tile([P, 1024], F32, tag="scr")
                    nc.vector.scalar_tensor_tensor(
                        out=scr[:], in0=ph[x][:], scalar=1.0,
                        in1=hid_sb[:, x * 1024:(x + 1) * 1024],
                        op0=ALU.mult, op1=ALU.mult,
                        accum_out=dt2[:, x:x + 1])
                u = pst.tile([P, 1], F32, tag="u")
                nc.vector.tensor_reduce(out=u[:], in_=sq2[:],
                                        axis=mybir.AxisListType.X, op=ALU.add)
                # u = sumsq + eps_k'  (rsq' = 1/sqrt(u) folds the /sqrt(H))
                nc.vector.tensor_scalar(out=u[:], in0=u[:],
                                        scalar1=consts[:, 0:1], scalar2=None,
                                        op0=ALU.add)
                rsq = newton_rsqrt(u)
                dot = pst.tile([P, 1], F32, tag="dot")
                nc.vector.tensor_reduce(out=dot[:], in_=dt2[:],
                                        axis=mybir.AxisListType.X, op=ALU.add)
                # gate = sigmoid(dot * rsq' - 4)
                nc.scalar.activation(out=sg_all[:, i:i + 1], in_=dot[:],
                                     func=AF.Sigmoid, scale=rsq[:],
                                     bias=consts[:, 3:4])

            with (
                tc.tile_pool(name="pWk", bufs=1) as pWk,
                tc.tile_pool(name="phid", bufs=3) as phid,
                tc.tile_pool(name="pkx", bufs=1, space="PSUM") as pkx,
            ):
                wkhi_sb = pWk.tile([P, HEADS, 2, H], FP8)
                wklo_sb = pWk.tile([P, HEADS, 2, H], FP8)
                for h in range(HEADS):
                    nc.sync.dma_start(out=wkhi_sb[:, h], in_=t_wkhi[:, h])
                # Wk_lo on the Pool queue, sequenced after the range-0/1
                # gathers so the early tiles' gathers win the DMA FIFO
                for h in range(HEADS):
                    nc.gpsimd.dma_start(out=wklo_sb[:, h], in_=t_wklo[:, h])
                emit_gathers(range(2, NRANGE))

                # tiles 0 and 1 are software-pipelined term-wise: their Whi
                # terms run while Wk_lo is still streaming in
                hid01 = []
                ph01 = []
                for i in range(2):
                    hid_sb = phid.tile([P, H], BF16, tag="hid")
                    nc.sync.dma_start(out=hid_sb[:],
                                      in_=t_hid[i * P:(i + 1) * P, :])
                    hid01.append(hid_sb)
                ph01.append([pbig.tile([P, 1024], F32, tag="ps", name="k0_0"),
                             pbig.tile([P, 1024], F32, tag="ps", name="k0_1")])
                ph01.append([pbig.tile([P, 1024], F32, tag="ps", name="k1_0"),
                             pkx.tile([P, 1024], F32, name="k1_1")])
                gemm_terms(0, wkhi_sb, wklo_sb, ph01[0], (0, 1), True, False)
                gemm_terms(1, wkhi_sb, wklo_sb, ph01[1], (0, 1), True, False)
                gemm_terms(0, wkhi_sb, wklo_sb, ph01[0], (2,), False, True)
                key_epi(0, ph01[0], hid01[0])
                gemm_terms(1, wkhi_sb, wklo_sb, ph01[1], (2,), False, True)
                key_epi(1, ph01[1], hid01[1])

                for i in range(2, NT):
                    if i == 8:
                        # mid-key prefetch of Wv_lo (Pool queue; DMA engines
                        # are free of startup traffic by now)
                        for h in range(HEADS):
                            nc.gpsimd.dma_start(out=wvlo_sb[:, h],
                                                in_=t_wvlo[:, h])
                    hid_sb = phid.tile([P, H], BF16, tag="hid")
                    nc.sync.dma_start(out=hid_sb[:],
                                      in_=t_hid[i * P:(i + 1) * P, :])
                    ph = [pbig.tile([P, 1024], F32, tag="ps", name=f"k{i}_{x}")
                          for x in range(2)]
                    gemm_terms(i, wkhi_sb, wklo_sb, ph, (0, 1, 2), True, True)
                    key_epi(i, ph, hid_sb)

            # ================= PHASE 2: value =================
            with (
                tc.tile_pool(name="pWvhi", bufs=1) as pWvhi,
                tc.tile_pool(name="pg", bufs=2) as pg,
                tc.tile_pool(name="pout", bufs=2) as pout,
                tc.tile_pool(name="pb", bufs=1) as pb,
                tc.tile_pool(name="pconv", bufs=2, space="PSUM") as pconv,
            ):
                # Wv_hi streamed on SP first; the small constant tiles after
                # it so they don't steal DMA-engine slots from the stream
                wvhi_sb = pWvhi.tile([P, HEADS, 2, H], FP8)
                for h in range(HEADS):
                    nc.sync.dma_start(out=wvhi_sb[:, h], in_=t_wvhi[:, h])
                w2p1_sb = pWvhi.tile([P, H], BF16)
                w1s_sb = pWvhi.tile([P, H], BF16)
                w0s_sb = pWvhi.tile([P, H], BF16)
                nc.sync.dma_start(out=w2p1_sb[:], in_=t_w2p1[:])
                nc.sync.dma_start(out=w1s_sb[:], in_=t_w1s[:])
                nc.sync.dma_start(out=w0s_sb[:], in_=t_w0s[:])
                nc.sync.dma_start(out=s12_sb[:], in_=t_s12[:])
                nc.sync.dma_start(out=eb_sb[:], in_=t_eb[:])

                # boundary ping-pong tiles (fixed, fully memset once so reads
                # of untouched rows are well-defined); tile 0's rows from host
                b_tiles = [pb.tile([P, 2, H], FP8, name=f"b{x}")
                           for x in range(2)]
                nc.vector.memset(b_tiles[0][:], 0.0)
                nc.vector.memset(b_tiles[1][:], 0.0)
                nc.sync.dma_start(out=b_tiles[0][0:4, 0, :], in_=t_bh[:])

                for i in range(NT):
                    b_prev = b_tiles[i % 2]
                    ph = [pbig.tile([P, 1024], F32, tag="ps", name=f"v{i}_{x}")
                          for x in range(2)]
                    gemm_terms(i, wvhi_sb, wvlo_sb, ph, (2, 0, 1), True, True)
                    sq2 = pst.tile([P, 2], F32, tag="sq2")
                    scr = pscr.tile([P, 1024], F32, tag="scr")
                    for x in range(2):
                        nc.scalar.activation(out=scr[:], in_=ph[x][:],
                                             func=AF.Square,
                                             accum_out=sq2[:, x:x + 1])
                    u = pst.tile([P, 1], F32, tag="u")
                    nc.vector.tensor_reduce(out=u[:], in_=sq2[:],
                                            axis=mybir.AxisListType.X,
                                            op=ALU.add)
                    nc.vector.tensor_scalar(out=u[:], in0=u[:],
                                            scalar1=consts[:, 1:2], scalar2=None,
                                            op0=ALU.add)
                    rsq = newton_rsqrt(u)
                    # s_final = rsq * sqrt(H) * gate
                    sfin = pst.tile([P, 1], F32, tag="sfin")
                    nc.vector.scalar_tensor_tensor(
                        out=sfin[:], in0=rsq[:], scalar=consts[:, 2:3],
                        in1=sg_all[:, i:i + 1], op0=ALU.mult, op1=ALU.mult)
                    # per-half epilogue pipeline: g -> g2/g10 -> conv -> copy
                    # -> add -> out DMA, so half 0 drains while half 1 is
                    # still in its GEMM/stat stages (shortens the end tail)
                    g = pg.tile([P, H], BF16, tag="g")
                    g2 = pg.tile([P, H], BF16, tag="g2")
                    g10 = pg.tile([P, 2, H], FP8, tag="g10")
                    out_sb = pout.tile([P, H], BF16, tag="out")
                    for x in range(2):
                        hs = slice(x * 1024, (x + 1) * 1024)
                        nc.scalar.activation(out=g[:, hs], in_=ph[x][:],
                                             func=AF.Copy, scale=sfin[:])
                        nc.vector.tensor_mul(out=g10[:, 0, hs], in0=g[:, hs],
                                             in1=w1s_sb[:, hs])
                        nc.vector.tensor_mul(out=g10[:, 1, hs], in0=g[:, hs],
                                             in1=w0s_sb[:, hs])
                        nc.vector.tensor_mul(out=g2[:, hs], in0=g[:, hs],
                                             in1=w2p1_sb[:, hs])
                        for ch in (2 * x, 2 * x + 1):
                            cs = slice(ch * 512, (ch + 1) * 512)
                            pcv = pconv.tile([P, 512], F32, tag="pcv")
                            nc.tensor.matmul(pcv[:], lhsT=s12_sb[:],
                                             rhs=g10[:, :, cs],
                                             start=True, stop=False,
                                             perf_mode=DR)
                            nc.tensor.matmul(pcv[:], lhsT=eb_sb[:],
                                             rhs=b_prev[:, :, cs],
                                             start=False, stop=True,
                                             perf_mode=DR)
                            nc.scalar.activation(out=out_sb[:, cs], in_=pcv[:],
                                                 func=AF.Copy)
                        nc.vector.tensor_add(out=out_sb[:, hs],
                                             in0=out_sb[:, hs],
                                             in1=g2[:, hs])
                        nc.sync.dma_start(out=t_out[i * P:(i + 1) * P, hs],
                                          in_=out_sb[:, hs])

                    # boundary rows for next tile from g10 tails
                    if i < NT - 1:
                        b_next = b_tiles[(i + 1) % 2]
                        nc.sync.dma_start(out=b_next[0:1, 0, :],
                                          in_=g10[127:128, 0, :])
                        nc.sync.dma_start(out=b_next[1:3, 0, :],
                                          in_=g10[126:128, 1, :])

    nc.compile()
    return nc


# ---------------------------------------------------------------- host wrapper
_PROGRAM = None


def _get_program():
    global _PROGRAM
    if _PROGRAM is None:
        _PROGRAM = build_program()
    return _PROGRAM


def kernel(hidden_states, input_ids, tables, Wk, Wv, key_norm_w, value_norm_w,
           conv_w, hm2, ho2, hm3, ho3):
    hidden_states = np.asarray(hidden_states, dtype=np.float32)
    input_ids = np.asarray(input_ids, dtype=np.int64)
    tables = np.asarray(tables, dtype=np.float32)
    Wk = np.asarray(Wk, dtype=np.float32)
    Wv = np.asarray(Wv, dtype=np.float32)
    key_norm_w = np.asarray(key_norm_w, dtype=np.float32)
    value_norm_w = np.asarray(value_norm_w, dtype=np.float32)
    conv_w = np.asarray(conv_w, dtype=np.float32)

    gidx = _global_indices(input_ids, np.asarray(hm2), np.asarray(ho2),
                           np.asarray(hm3), np.asarray(ho3))   # [B,S,8] i32
    hid_local = (gidx % VOCAB).reshape(B * S, HEADS)           # per-head rows

    # fold key_norm into hidden; fp8 scales
    hid_flat = (hidden_states.reshape(B * S, H) * key_norm_w[None, :])
    hid_bf = hid_flat.astype(BFNP)

    sM = FMAX / max(np.abs(tables).max(), 1e-30)
    sWk = FMAX / max(np.abs(Wk).max(), 1e-30)
    sWv = FMAX / max(np.abs(Wv).max(), 1e-30)

    def split_fp8(x, s):
        hi = (x * s).astype(E4)
        lo = ((x * s) - hi.astype(np.float32)).astype(E4)
        return hi, lo

    def w_layout(W, s):
        # [p, head, c, n] with W.T[k, n] = W[n, k]; k = h*256 + c*128 + p
        hi, lo = split_fp8(np.ascontiguousarray(W.T), s)   # [k, n]
        def lay(a):
            return np.ascontiguousarray(
                a.reshape(HEADS, 2, P, H).transpose(2, 0, 1, 3))
        return lay(hi), lay(lo)

    wkhi, wklo = w_layout(Wk, sWk)
    wvhi, wvlo = w_layout(Wv, sWv)

    # conv weight foldings (value_norm + fp8 g-scale)
    w0 = conv_w[:, 0] * value_norm_w
    w1 = conv_w[:, 1] * value_norm_w
    w2p1 = (1.0 + conv_w[:, 2]) * value_norm_w
    w2p1bc = np.ascontiguousarray(np.broadcast_to(w2p1, (P, H))).astype(BFNP)
    w1sbc = np.ascontiguousarray(np.broadcast_to(w1 * SG, (P, H))).astype(BFNP)
    w0sbc = np.ascontiguousarray(np.broadcast_to(w0 * SG, (P, H))).astype(BFNP)

    # shift-pair (s1, s2) and boundary lhsT matrices, scaled by 1/SG
    inv = np.float32(1.0 / SG)
    s12 = np.zeros((P, 2, P), E4)
    s12[:, 0, :] = (np.eye(P, k=1, dtype=np.float32) * inv).astype(E4)
    s12[:, 1, :] = (np.eye(P, k=2, dtype=np.float32) * inv).astype(E4)
    eb = np.zeros((P, 2, P), E4)
    # b_pad rows: 0 -> out0 (g1[127]), 1 -> out0 (g0[126]), 2 -> out1 (g0[127])
    eb[0, 0, 0] = E4(inv)
    eb[1, 0, 0] = E4(inv)
    eb[2, 0, 1] = E4(inv)

    consts = np.zeros((P, 4), np.float32)
    consts[:, 0] = (sM * sWk) ** 2 * H * EPS
    consts[:, 1] = (sM * sWv) ** 2 * H * EPS
    consts[:, 2] = np.sqrt(np.float32(H))
    consts[:, 3] = GATE_BIAS

    # exact host reference for the 2 boundary tokens of each core
    def host_gated(trange):
        """gated[t] rows (f64->f32) for global token indices trange."""
        out = np.zeros((len(trange), H), np.float32)
        tabs = tables.astype(np.float64)
        for j, t in enumerate(trange):
            rows = [tabs[h, hid_local[t, h]] for h in range(HEADS)]
            mem = np.concatenate(rows)                     # [2048]
            kr = mem @ Wk.T.astype(np.float64)
            vr = mem @ Wv.T.astype(np.float64)
            rk = 1.0 / np.sqrt(np.mean(kr ** 2) + EPS)
            rv = 1.0 / np.sqrt(np.mean(vr ** 2) + EPS)
            z = float(hid_flat[t].astype(np.float64) @ (kr * rk)) / np.sqrt(H) \
                + GATE_BIAS
            gate = 1.0 / (1.0 + np.exp(-z))
            out[j] = (gate * (vr * rv) * value_norm_w).astype(np.float32)
        return out

    in_maps = []
    for r in range(N_CORES):
        t0 = r * TOK
        idx_core = hid_local[t0:t0 + TOK]                  # [2048, 8]
        in_map = {}
        packed = np.zeros((HEADS * CROWS, 512), np.uint8)
        gidx16 = np.empty((TOK, HEADS), np.int16)          # h*2048 + inv
        for h in range(HEADS):
            uniq, invmap = np.unique(idx_core[:, h], return_inverse=True)
            rows = tables[h, uniq]                         # [n_u, 256]
            hi, lo = split_fp8(rows, sM)
            blk = packed[h * CROWS:h * CROWS + len(uniq)]
            blk[:, 0::2] = hi.view(np.uint8)
            blk[:, 1::2] = lo.view(np.uint8)
            gidx16[:, h] = (h * CROWS + invmap).astype(np.int16)
        in_map["ctab"] = packed.view(E4)
        # gather idx rectangles: (token-range r, head-group hg): the 1024
        # idxs are head-major (4 heads x 256 tokens) to match m_slabs
        idx_cols = []
        for r_ in range(NRANGE):
            for hg in range(NHG):
                hpg = HEADS // NHG
                rect = gidx16[r_ * GTOK:(r_ + 1) * GTOK,
                              hg * hpg:(hg + 1) * hpg]
                seq = np.ascontiguousarray(rect.T).reshape(-1)   # head-major
                idx_cols.append(_wrap_idx(seq, hpg * GTOK))
        in_map["idx0"] = np.ascontiguousarray(
            np.concatenate(idx_cols[:NHG], axis=1))
        in_map["idxr"] = np.ascontiguousarray(
            np.concatenate(idx_cols[NHG:], axis=1))

        # boundary rows for tile 0
        bh = np.zeros((4, H), np.float32)
        if t0 % S != 0:
            gtwo = host_gated([t0 - 1, t0 - 2])            # [2, H]
            bh[0] = gtwo[0] * w1 * SG                      # g1[t0-1]
            bh[1] = gtwo[1] * w0 * SG                      # g0[t0-2]
            bh[2] = gtwo[0] * w0 * SG                      # g0[t0-1]
        in_map["bhost"] = bh.astype(E4)

        in_map.update({
            "wkhi": wkhi, "wklo": wklo, "wvhi": wvhi, "wvlo": wvlo,
            "hid": np.ascontiguousarray(hid_bf[t0:t0 + TOK]),
            "w2p1bc": w2p1bc, "w1sbc": w1sbc, "w0sbc": w0sbc,
            "s12": s12, "eb": eb, "consts": consts,
        })
        in_maps.append(in_map)

    nc = _get_program()
    res = run_bass_kernel_spmd(nc, in_maps, list(range(N_CORES)))
    out = np.empty((B * S, H), np.float32)
    for r in range(N_CORES):
        out[r * TOK:(r + 1) * TOK] = res.results[r]["out"].astype(np.float32)
    return out.reshape(B, S, H)


# revision 56
# speedup vs baseline: 1.5886x; 1.0025x over previous
"""EngramModule kernel for Trainium2 (8 NeuronCores, SPMD data-parallel).

v2 architecture (fp8 DoubleRow 3-term GEMMs + transpose-gather):

Per token t (feature dim H=2048):
  idx[t, h]   = hash of n-gram ending at t (8 heads; computed on host)
  memory[t]   = concat_h tables[h, idx[t, h]]
  key_raw     = memory @ Wk.T ; value_raw = memory @ Wv.T
  gate        = sigmoid(dot(hidden, key_raw)/(sqrt(H)*rms_k) - 4)
  g[t]        = gate * value_raw / rms_v           (value_norm folded in conv w)
  out[t]      = g[t]*(1+w2) + w1*g[t-1] + w0*g[t-2]

Device strategy per core (2048 tokens, 16 tiles of 128):
 - Tables are compacted on host per (core, head) to <=2048 unique rows and
   packed as fp8 e4m3 (hi, lo) byte-interleaved 512B rows. One dma_gather
   with transpose=True per (head, segment) delivers memory ALREADY in lhsT
   layout [k-dim on partitions, tokens on free], with (hi, lo) as the two
   bytes of each 16-bit transpose unit -> directly usable as DoubleRow
   operand slots.
 - GEMMs run as 3-term compensated fp8 DoubleRow matmuls (error ~0.1%):
     (Mhi 2-slab)x(Whi 2-slab) + (Mlo)x(Whi) + (Mhi)x(Wlo)
   at 0.25 PE-cycles per output column per 128-contraction.
 - Phase 1 (key): per tile accumulate k in PSUM, ACT-square sumsq + DVE dot
   with bf16 hidden, Newton rsqrt, one ACT Sigmoid -> per-token gate.
 - Phase 2 (value): v in PSUM, sumsq -> rsq_v, g = ACT copy(v * s) bf16;
   g1/g0 cast to fp8; conv = one fp8-DR matmul (shift-pair slots) + one
   fp8-DR boundary matmul per 512-chunk; out = ACT copy(conv psum) + g2.
 - Conv boundary rows for tile 0 are computed EXACTLY on host (2 tokens).
 - Output written bf16, upcast on host.
Only ACT funcs {Square, Sigmoid, Copy} are used -> single act table set,
zero LoadActFuncSet reloads.
"""

import sys

import numpy as np

try:
    import concourse.bass as bass  # noqa: F401
except ImportError:
    sys.path.insert(0, "/opt/trn_rl_repo")

import ml_dtypes

import concourse.bacc as bacc
import concourse.bass as bass
import concourse.tile as tile
from concourse import mybir
from concourse.bass_utils import run_bass_kernel_spmd

E4 = ml_dtypes.float8_e4m3fn
BFNP = ml_dtypes.bfloat16
F32 = mybir.dt.float32
BF16 = mybir.dt.bfloat16
FP8 = mybir.dt.float8e4
I16 = mybir.dt.int16
I32 = mybir.dt.int32
ALU = mybir.AluOpType
AF = mybir.ActivationFunctionType
DR = mybir.MatmulPerfMode.DoubleRow

P = 128
H = 2048
HEADS = 8
HEAD_DIM = 256
VOCAB = 65536
MODULUS = VOCAB - 1
EPS = 1e-6
GATE_BIAS = -4.0
N_CORES = 8
B, S = 4, 4096
TOK = (B * S) // N_CORES        # 2048 tokens per core
NT = TOK // P                   # 16 tiles
CROWS = 2048                    # compact table rows per (core, head)
# gathers batch 4 heads x 256 tokens = 1024 indices (fits the 1024-slot
# SWDGE ring) against the concatenated per-core table [8*2048, 512]
GTOK = 256                      # tokens per gather
NRANGE = TOK // GTOK            # 8 token ranges
NHG = 4                         # head groups of 2
SG = 128.0                      # fp8 scale for g1/g0 (conv operands)
FMAX = 64.0                     # fp8 operand absmax (PSUM partial < ~5.5e4)
NCH = H // 512                  # 4 col chunks of 512


# ---------------------------------------------------------------- host hashing
def _hash_ids_np(ids, mult, off, n):
    """Exact replica of the reference _hash_ids in numpy (wrapping int64)."""
    Bb, Ss = ids.shape
    nh = mult.shape[0]
    ids_u = ids.astype(np.uint64)
    mult_u = mult.astype(np.uint64)
    off_u = off.astype(np.uint64)
    mix = np.zeros((Bb, Ss, nh), dtype=np.uint64)
    for p in range(n):
        shift = n - 1 - p
        tok = np.zeros_like(ids_u)
        if shift > 0:
            tok[:, shift:] = ids_u[:, : Ss - shift]
        else:
            tok = ids_u
        mix ^= tok[:, :, None] * mult_u[None, None, :, p]
    h = (mix + off_u[None, None, :]).view(np.int64)
    hmod = np.remainder(h, MODULUS) + 1
    valid = (np.arange(Ss) >= n - 1)[None, :, None]
    return np.where(valid, hmod, 0)


def _global_indices(input_ids, hm2, ho2, hm3, ho3):
    """[B, S, 8] int32 row indices into the flattened [8*65536, 256] table."""
    h2 = _hash_ids_np(input_ids, hm2, ho2, 2)
    h3 = _hash_ids_np(input_ids, hm3, ho3, 3)
    hid = np.concatenate([h2, h3], axis=-1)          # [B, S, 8]
    gidx = hid + (np.arange(HEADS, dtype=np.int64) * VOCAB)[None, None, :]
    return gidx.astype(np.int32)


def _wrap_idx(inv, n_tok):
    """int16 idx tile [128, n_tok//16]: slot i lives at [i%16, i//16]."""
    t = np.zeros((16, n_tok // 16), np.int16)
    t[np.arange(n_tok) % 16, np.arange(n_tok) // 16] = inv.astype(np.int16)
    return np.ascontiguousarray(np.tile(t, (8, 1)))


# ---------------------------------------------------------------- device program
def build_program():
    nc = bacc.Bacc(None, target_bir_lowering=False)

    t_ctab = nc.dram_tensor("ctab", [HEADS * CROWS, 512], FP8,
                            kind="ExternalInput")
    # idx tiles batched into two tensors: tiny range-0 block loads first so
    # the first gathers start immediately. 64 int16 words per gather.
    IDXW0 = (HEADS // NHG * GTOK) // 16 * NHG
    IDXWR = (HEADS // NHG * GTOK) // 16 * NHG * (NRANGE - 1)
    t_idx0 = nc.dram_tensor("idx0", [P, IDXW0], I16, kind="ExternalInput")
    t_idxr = nc.dram_tensor("idxr", [P, IDXWR], I16, kind="ExternalInput")
    t_wkhi = nc.dram_tensor("wkhi", [P, HEADS, 2, H], FP8, kind="ExternalInput")
    t_wklo = nc.dram_tensor("wklo", [P, HEADS, 2, H], FP8, kind="ExternalInput")
    t_wvhi = nc.dram_tensor("wvhi", [P, HEADS, 2, H], FP8, kind="ExternalInput")
    t_wvlo = nc.dram_tensor("wvlo", [P, HEADS, 2, H], FP8, kind="ExternalInput")
    t_hid = nc.dram_tensor("hid", [TOK, H], BF16, kind="ExternalInput")
    t_w2p1 = nc.dram_tensor("w2p1bc", [P, H], BF16, kind="ExternalInput")
    t_w1s = nc.dram_tensor("w1sbc", [P, H], BF16, kind="ExternalInput")
    t_w0s = nc.dram_tensor("w0sbc", [P, H], BF16, kind="ExternalInput")
    t_s12 = nc.dram_tensor("s12", [P, 2, P], FP8, kind="ExternalInput")
    t_eb = nc.dram_tensor("eb", [P, 2, P], FP8, kind="ExternalInput")
    t_bh = nc.dram_tensor("bhost", [4, H], FP8, kind="ExternalInput")
    t_out = nc.dram_tensor("out", [TOK, H], BF16, kind="ExternalOutput")

    # scalar constants (host-computed, folded scales)
    t_consts = nc.dram_tensor("consts", [P, 4], F32, kind="ExternalInput")
    # consts columns: 0 = eps_k' = (sM*sWk)^2 * H * EPS
    #                 1 = eps_v' = (sM*sWv)^2 * H * EPS
    #                 2 = sqrt(H)
    #                 3 = GATE_BIAS

    with tile.TileContext(nc) as tc:
        with (
            tc.tile_pool(name="pconst", bufs=1) as pc,
            tc.tile_pool(name="pM", bufs=1) as pM,
            tc.tile_pool(name="pWvpre", bufs=1) as pWvpre,
            tc.tile_pool(name="pstat", bufs=4) as pst,
            tc.tile_pool(name="pscr", bufs=2) as pscr,
            tc.tile_pool(name="pbig", bufs=3, space="PSUM") as pbig,
        ):
            # ---- small consts
            idx0_sb = pc.tile([P, IDXW0], I16)
            nc.sync.dma_start(out=idx0_sb[:], in_=t_idx0[:])
            consts = pc.tile([P, 4], F32)
            nc.sync.dma_start(out=consts[:], in_=t_consts[:])
            idxr_sb = pc.tile([P, IDXWR], I16)
            nc.sync.dma_start(out=idxr_sb[:], in_=t_idxr[:])
            s12_sb = pc.tile([P, 2, P], FP8)
            eb_sb = pc.tile([P, 2, P], FP8)
            sg_all = pc.tile([P, NT], F32)      # per-tile gate scalars

            # ---- gathers: (token-range, head-group) rectangles of 1024 idx
            HPG = HEADS // NHG  # heads per gather
            NIG = HPG * GTOK    # idxs per gather
            mseg = [[None] * NHG for _ in range(NRANGE)]

            def emit_gathers(ranges):
                for r in ranges:
                    for hg in range(NHG):
                        g = r * NHG + hg
                        W = NIG // 16
                        if r == 0:
                            iap = idx0_sb[:, hg * W:(hg + 1) * W]
                        else:
                            w0 = (g - NHG) * W
                            iap = idxr_sb[:, w0:w0 + W]
                        m = pM.tile([P, 4 * NIG], FP8, name=f"m{r}_{hg}")
                        nc.gpsimd.dma_gather(
                            out_ap=m[:].rearrange("p (f t) -> p f t", f=4),
                            in_ap=t_ctab[:],
                            idxs_ap=iap,
                            num_idxs=NIG, num_idxs_reg=NIG,
                            elem_size=512, transpose=True)
                        mseg[r][hg] = m

            emit_gathers(range(2))

            def m_slabs(h, i):
                """(lhsT_hi, lhsT_lo) [p, c(2), t(128)] for tile i, head h."""
                m = mseg[i // 2][h // (HEADS // NHG)]
                t0 = (h % (HEADS // NHG)) * GTOK + (i % 2) * P
                ctj = m[:].rearrange("p (c t j) -> p c t j", c=2, t=NIG, j=2)
                return (ctj[:, :, t0:t0 + P, 0], ctj[:, :, t0:t0 + P, 1])

            def gemm_terms(i, whi_sb, wlo_sb, ph, terms, start, stop,
                           chunks=tuple(range(NCH))):
                """Emit a subset of the 3-term fp8 DR GEMM for tile i into
                psum halves ph[0|1]. Terms: 0 = Mhi@Whi, 1 = Mlo@Whi,
                2 = Mhi@Wlo. start/stop apply at the first/last emitted
                matmul of each psum chunk group."""
                for tx, term in enumerate(terms):
                    for h in range(HEADS):
                        hi, lo = m_slabs(h, i)
                        mop = hi if term != 1 else lo
                        wsb = whi_sb if term != 2 else wlo_sb
                        for ch in chunks:
                            pt = ph[ch // 2]
                            cs = slice((ch % 2) * 512, (ch % 2) * 512 + 512)
                            nc.tensor.matmul(
                                pt[:, cs], lhsT=mop,
                                rhs=wsb[:, h, :, ch * 512:(ch + 1) * 512],
                                start=(start and tx == 0 and h == 0),
                                stop=(stop and tx == len(terms) - 1
                                      and h == HEADS - 1),
                                perf_mode=DR)

            def newton_rsqrt(u):
                """In-place u <- 1/sqrt(u) via bit-seed + 2 Newton iters."""
                y = pst.tile([P, 1], F32, tag="ny")
                yi = y[:].bitcast(I32)
                nc.vector.tensor_scalar(out=yi, in0=u[:].bitcast(I32), scalar1=1,
                                        scalar2=None, op0=ALU.logical_shift_right)
                nc.vector.tensor_scalar(out=yi, in0=yi, scalar1=-1,
                                        scalar2=0x5F3759DF, op0=ALU.mult,
                                        op1=ALU.add)
                t2 = pst.tile([P, 1], F32, tag="nt")
                for _ in range(2):
                    nc.vector.tensor_mul(out=t2[:], in0=y[:], in1=y[:])
                    nc.vector.tensor_mul(out=t2[:], in0=t2[:], in1=u[:])
                    nc.vector.tensor_scalar(out=t2[:], in0=t2[:], scalar1=-0.5,
                                            scalar2=1.5, op0=ALU.mult, op1=ALU.add)
                    nc.vector.tensor_mul(out=y[:], in0=y[:], in1=t2[:])
                return y

            # ================= PHASE 1: key =================
            wvlo_sb = pWvpre.tile([P, HEADS, 2, H], FP8)

            def key_epi(i, ph, hid_sb):
                # sumsq + dot per half
                sq2 = pst.tile([P, 2], F32, tag="sq2")
                dt2 = pst.tile([P, 2], F32, tag="dt2")
                for x in range(2):
                    scr = pscr.tile([P, 1024], F32, tag="scr")
                    nc.scalar.activation(out=scr[:], in_=ph[x][:],
                                         func=AF.Square,
                                         accum_out=sq2[:, x:x + 1])
                    scr = pscr.tile([P, 1024], F32, tag="scr")
                    nc.vector.scalar_tensor_tensor(
                        out=scr[:], in0=ph[x][:], scalar=1.0,
                        in1=hid_sb[:, x * 1024:(x + 1) * 1024],
                        op0=ALU.mult, op1=ALU.mult,
                        accum_out=dt2[:, x:x + 1])
                u = pst.tile([P, 1], F32, tag="u")
                nc.vector.tensor_reduce(out=u[:], in_=sq2[:],
                                        axis=mybir.AxisListType.X, op=ALU.add)
                # u = sumsq + eps_k'  (rsq' = 1/sqrt(u) folds the /sqrt(H))
                nc.vector.tensor_scalar(out=u[:], in0=u[:],
                                        scalar1=consts[:, 0:1], scalar2=None,
                                        op0=ALU.add)
                rsq = newton_rsqrt(u)
                dot = pst.tile([P, 1], F32, tag="dot")
                nc.vector.tensor_reduce(out=dot[:], in_=dt2[:],
                                        axis=mybir.AxisListType.X, op=ALU.add)
                # gate = sigmoid(dot * rsq' - 4)
                nc.scalar.activation(out=sg_all[:, i:i + 1], in_=dot[:],
                                     func=AF.Sigmoid, scale=rsq[:],
                                     bias=consts[:, 3:4])

            with (
                tc.tile_pool(name="pWk", bufs=1) as pWk,
                tc.tile_pool(name="phid", bufs=3) as phid,
                tc.tile_pool(name="pkx", bufs=1, space="PSUM") as pkx,
            ):
                wkhi_sb = pWk.tile([P, HEADS, 2, H], FP8)
                wklo_sb = pWk.tile([P, HEADS, 2, H], FP8)
                for h in range(HEADS):
                    nc.sync.dma_start(out=wkhi_sb[:, h], in_=t_wkhi[:, h])
                # Wk_lo on the Pool queue, sequenced after the range-0/1
                # gathers so the early tiles' gathers win the DMA FIFO
                for h in range(HEADS):
                    nc.gpsimd.dma_start(out=wklo_sb[:, h], in_=t_wklo[:, h])
                emit_gathers(range(2, NRANGE))

                # tiles 0 and 1 are software-pipelined term-wise: their Whi
                # terms run while Wk_lo is still streaming in
                hid01 = []
                ph01 = []
                for i in range(2):
                    hid_sb = phid.tile([P, H], BF16, tag="hid")
                    nc.sync.dma_start(out=hid_sb[:],
                                      in_=t_hid[i * P:(i + 1) * P, :])
                    hid01.append(hid_sb)
                ph01.append([pbig.tile([P, 1024], F32, tag="ps", name="k0_0"),
                             pbig.tile([P, 1024], F32, tag="ps", name="k0_1")])
                ph01.append([pbig.tile([P, 1024], F32, tag="ps", name="k1_0"),
                             pkx.tile([P, 1024], F32, name="k1_1")])
                gemm_terms(0, wkhi_sb, wklo_sb, ph01[0], (0, 1), True, False)
                gemm_terms(1, wkhi_sb, wklo_sb, ph01[1], (0, 1), True, False)
                gemm_terms(0, wkhi_sb, wklo_sb, ph01[0], (2,), False, True)
                key_epi(0, ph01[0], hid01[0])
                gemm_terms(1, wkhi_sb, wklo_sb, ph01[1], (2,), False, True)
                key_epi(1, ph01[1], hid01[1])

                for i in range(2, NT):
                    if i == 8:
                        # mid-key prefetch of Wv_lo (Pool queue; DMA engines
                        # are free of startup traffic by now)
                        for h in range(HEADS):
                            nc.gpsimd.dma_start(out=wvlo_sb[:, h],
                                                in_=t_wvlo[:, h])
                    hid_sb = phid.tile([P, H], BF16, tag="hid")
                    nc.sync.dma_start(out=hid_sb[:],
                                      in_=t_hid[i * P:(i + 1) * P, :])
                    ph = [pbig.tile([P, 1024], F32, tag="ps", name=f"k{i}_{x}")
                          for x in range(2)]
                    gemm_terms(i, wkhi_sb, wklo_sb, ph, (0, 1, 2), True, True)
                    key_epi(i, ph, hid_sb)

            # ================= PHASE 2: value =================
            with (
                tc.tile_pool(name="pWvhi", bufs=1) as pWvhi,
                tc.tile_pool(name="pg", bufs=2) as pg,
                tc.tile_pool(name="pout", bufs=2) as pout,
                tc.tile_pool(name="pb", bufs=1) as pb,
                tc.tile_pool(name="pconv", bufs=2, space="PSUM") as pconv,
            ):
                # Wv_hi streamed on SP first; the small constant tiles after
                # it so they don't steal DMA-engine slots from the stream
                wvhi_sb = pWvhi.tile([P, HEADS, 2, H], FP8)
                for h in range(HEADS):
                    nc.sync.dma_start(out=wvhi_sb[:, h], in_=t_wvhi[:, h])
                w2p1_sb = pWvhi.tile([P, H], BF16)
                w1s_sb = pWvhi.tile([P, H], BF16)
                w0s_sb = pWvhi.tile([P, H], BF16)
                nc.sync.dma_start(out=w2p1_sb[:], in_=t_w2p1[:])
                nc.sync.dma_start(out=w1s_sb[:], in_=t_w1s[:])
                nc.sync.dma_start(out=w0s_sb[:], in_=t_w0s[:])
                nc.sync.dma_start(out=s12_sb[:], in_=t_s12[:])
                nc.sync.dma_start(out=eb_sb[:], in_=t_eb[:])

                # boundary ping-pong tiles (fixed, fully memset once so reads
                # of untouched rows are well-defined); tile 0's rows from host
                b_tiles = [pb.tile([P, 2, H], FP8, name=f"b{x}")
                           for x in range(2)]
                nc.vector.memset(b_tiles[0][:], 0.0)
                nc.vector.memset(b_tiles[1][:], 0.0)
                nc.sync.dma_start(out=b_tiles[0][0:4, 0, :], in_=t_bh[:])

                for i in range(NT):
                    b_prev = b_tiles[i % 2]
                    ph = [pbig.tile([P, 1024], F32, tag="ps", name=f"v{i}_{x}")
                          for x in range(2)]
                    sq2 = pst.tile([P, 2], F32, tag="sq2")
                    # half-major: half 0's sumsq overlaps half 1's GEMM
                    for x in range(2):
                        gemm_terms(i, wvhi_sb, wvlo_sb, ph, (2, 0, 1),
                                   True, True, chunks=(2 * x, 2 * x + 1))
                        scr = pscr.tile([P, 1024], F32, tag="scr")
                        nc.scalar.activation(out=scr[:], in_=ph[x][:],
                                             func=AF.Square,
                                             accum_out=sq2[:, x:x + 1])
                    u = pst.tile([P, 1], F32, tag="u")
                    nc.vector.tensor_reduce(out=u[:], in_=sq2[:],
                                            axis=mybir.AxisListType.X,
                                            op=ALU.add)
                    nc.vector.tensor_scalar(out=u[:], in0=u[:],
                                            scalar1=consts[:, 1:2], scalar2=None,
                                            op0=ALU.add)
                    rsq = newton_rsqrt(u)
                    # s_final = rsq * sqrt(H) * gate
                    sfin = pst.tile([P, 1], F32, tag="sfin")
                    nc.vector.scalar_tensor_tensor(
                        out=sfin[:], in0=rsq[:], scalar=consts[:, 2:3],
                        in1=sg_all[:, i:i + 1], op0=ALU.mult, op1=ALU.mult)
                    # per-half epilogue pipeline: g -> g2/g10 -> conv -> copy
                    # -> add -> out DMA, so half 0 drains while half 1 is
                    # still in its GEMM/stat stages (shortens the end tail)
                    g = pg.tile([P, H], BF16, tag="g")
                    g2 = pg.tile([P, H], BF16, tag="g2")
                    g10 = pg.tile([P, 2, H], FP8, tag="g10")
                    out_sb = pout.tile([P, H], BF16, tag="out")
                    for x in range(2):
                        hs = slice(x * 1024, (x + 1) * 1024)
                        nc.scalar.activation(out=g[:, hs], in_=ph[x][:],
                                             func=AF.Copy, scale=sfin[:])
                        nc.vector.tensor_mul(out=g10[:, 0, hs], in0=g[:, hs],
                                             in1=w1s_sb[:, hs])
                        nc.vector.tensor_mul(out=g10[:, 1, hs], in0=g[:, hs],
                                             in1=w0s_sb[:, hs])
                        nc.vector.tensor_mul(out=g2[:, hs], in0=g[:, hs],
                                             in1=w2p1_sb[:, hs])
                        for ch in (2 * x, 2 * x + 1):
                            cs = slice(ch * 512, (ch + 1) * 512)
                            pcv = pconv.tile([P, 512], F32, tag="pcv")
                            nc.tensor.matmul(pcv[:], lhsT=s12_sb[:],
                                             rhs=g10[:, :, cs],
                                             start=True, stop=False,
                                             perf_mode=DR)
                            nc.tensor.matmul(pcv[:], lhsT=eb_sb[:],
                                             rhs=b_prev[:, :, cs],
                                             start=False, stop=True,
                                             perf_mode=DR)
                            nc.scalar.activation(out=out_sb[:, cs], in_=pcv[:],
                                                 func=AF.Copy)
                        nc.vector.tensor_add(out=out_sb[:, hs],
                                             in0=out_sb[:, hs],
                                             in1=g2[:, hs])
                        nc.sync.dma_start(out=t_out[i * P:(i + 1) * P, hs],
                                          in_=out_sb[:, hs])

                    # boundary rows for next tile from g10 tails
                    if i < NT - 1:
                        b_next = b_tiles[(i + 1) % 2]
                        nc.sync.dma_start(out=b_next[0:1, 0, :],
                                          in_=g10[127:128, 0, :])
                        nc.sync.dma_start(out=b_next[1:3, 0, :],
                                          in_=g10[126:128, 1, :])

    nc.compile()
    return nc


# ---------------------------------------------------------------- host wrapper
_PROGRAM = None


def _get_program():
    global _PROGRAM
    if _PROGRAM is None:
        _PROGRAM = build_program()
    return _PROGRAM


def kernel(hidden_states, input_ids, tables, Wk, Wv, key_norm_w, value_norm_w,
           conv_w, hm2, ho2, hm3, ho3):
    hidden_states = np.asarray(hidden_states, dtype=np.float32)
    input_ids = np.asarray(input_ids, dtype=np.int64)
    tables = np.asarray(tables, dtype=np.float32)
    Wk = np.asarray(Wk, dtype=np.float32)
    Wv = np.asarray(Wv, dtype=np.float32)
    key_norm_w = np.asarray(key_norm_w, dtype=np.float32)
    value_norm_w = np.asarray(value_norm_w, dtype=np.float32)
    conv_w = np.asarray(conv_w, dtype=np.float32)

    gidx = _global_indices(input_ids, np.asarray(hm2), np.asarray(ho2),
                           np.asarray(hm3), np.asarray(ho3))   # [B,S,8] i32
    hid_local = (gidx % VOCAB).reshape(B * S, HEADS)           # per-head rows

    # fold key_norm into hidden; fp8 scales
    hid_flat = (hidden_states.reshape(B * S, H) * key_norm_w[None, :])
    hid_bf = hid_flat.astype(BFNP)

    sM = FMAX / max(np.abs(tables).max(), 1e-30)
    sWk = FMAX / max(np.abs(Wk).max(), 1e-30)
    sWv = FMAX / max(np.abs(Wv).max(), 1e-30)

    def split_fp8(x, s):
        hi = (x * s).astype(E4)
        lo = ((x * s) - hi.astype(np.float32)).astype(E4)
        return hi, lo

    def w_layout(W, s):
        # [p, head, c, n] with W.T[k, n] = W[n, k]; k = h*256 + c*128 + p
        hi, lo = split_fp8(np.ascontiguousarray(W.T), s)   # [k, n]
        def lay(a):
            return np.ascontiguousarray(
                a.reshape(HEADS, 2, P, H).transpose(2, 0, 1, 3))
        return lay(hi), lay(lo)

    wkhi, wklo = w_layout(Wk, sWk)
    wvhi, wvlo = w_layout(Wv, sWv)

    # conv weight foldings (value_norm + fp8 g-scale)
    w0 = conv_w[:, 0] * value_norm_w
    w1 = conv_w[:, 1] * value_norm_w
    w2p1 = (1.0 + conv_w[:, 2]) * value_norm_w
    w2p1bc = np.ascontiguousarray(np.broadcast_to(w2p1, (P, H))).astype(BFNP)
    w1sbc = np.ascontiguousarray(np.broadcast_to(w1 * SG, (P, H))).astype(BFNP)
    w0sbc = np.ascontiguousarray(np.broadcast_to(w0 * SG, (P, H))).astype(BFNP)

    # shift-pair (s1, s2) and boundary lhsT matrices, scaled by 1/SG
    inv = np.float32(1.0 / SG)
    s12 = np.zeros((P, 2, P), E4)
    s12[:, 0, :] = (np.eye(P, k=1, dtype=np.float32) * inv).astype(E4)
    s12[:, 1, :] = (np.eye(P, k=2, dtype=np.float32) * inv).astype(E4)
    eb = np.zeros((P, 2, P), E4)
    # b_pad rows: 0 -> out0 (g1[127]), 1 -> out0 (g0[126]), 2 -> out1 (g0[127])
    eb[0, 0, 0] = E4(inv)
    eb[1, 0, 0] = E4(inv)
    eb[2, 0, 1] = E4(inv)

    consts = np.zeros((P, 4), np.float32)
    consts[:, 0] = (sM * sWk) ** 2 * H * EPS
    consts[:, 1] = (sM * sWv) ** 2 * H * EPS
    consts[:, 2] = np.sqrt(np.float32(H))
    consts[:, 3] = GATE_BIAS

    # exact host reference for the 2 boundary tokens of each core
    def host_gated(trange):
        """gated[t] rows (f64->f32) for global token indices trange."""
        out = np.zeros((len(trange), H), np.float32)
        tabs = tables.astype(np.float64)
        for j, t in enumerate(trange):
            rows = [tabs[h, hid_local[t, h]] for h in range(HEADS)]
            mem = np.concatenate(rows)                     # [2048]
            kr = mem @ Wk.T.astype(np.float64)
            vr = mem @ Wv.T.astype(np.float64)
            rk = 1.0 / np.sqrt(np.mean(kr ** 2) + EPS)
            rv = 1.0 / np.sqrt(np.mean(vr ** 2) + EPS)
            z = float(hid_flat[t].astype(np.float64) @ (kr * rk)) / np.sqrt(H) \
                + GATE_BIAS
            gate = 1.0 / (1.0 + np.exp(-z))
            out[j] = (gate * (vr * rv) * value_norm_w).astype(np.float32)
        return out

    in_maps = []
    for r in range(N_CORES):
        t0 = r * TOK
        idx_core = hid_local[t0:t0 + TOK]                  # [2048, 8]
        in_map = {}
        packed = np.zeros((HEADS * CROWS, 512), np.uint8)
        gidx16 = np.empty((TOK, HEADS), np.int16)          # h*2048 + inv
        for h in range(HEADS):
            uniq, invmap = np.unique(idx_core[:, h], return_inverse=True)
            rows = tables[h, uniq]                         # [n_u, 256]
            hi, lo = split_fp8(rows, sM)
            blk = packed[h * CROWS:h * CROWS + len(uniq)]
            blk[:, 0::2] = hi.view(np.uint8)
            blk[:, 1::2] = lo.view(np.uint8)
            gidx16[:, h] = (h * CROWS + invmap).astype(np.int16)
        in_map["ctab"] = packed.view(E4)
        # gather idx rectangles: (token-range r, head-group hg): the 1024
        # idxs are head-major (4 heads x 256 tokens) to match m_slabs
        idx_cols = []
        for r_ in range(NRANGE):
            for hg in range(NHG):
                hpg = HEADS // NHG
                rect = gidx16[r_ * GTOK:(r_ + 1) * GTOK,
                              hg * hpg:(hg + 1) * hpg]
                seq = np.ascontiguousarray(rect.T).reshape(-1)   # head-major
                idx_cols.append(_wrap_idx(seq, hpg * GTOK))
        in_map["idx0"] = np.ascontiguousarray(
            np.concatenate(idx_cols[:NHG], axis=1))
        in_map["idxr"] = np.ascontiguousarray(
            np.concatenate(idx_cols[NHG:], axis=1))

        # boundary rows for tile 0
        bh = np.zeros((4, H), np.float32)
        if t0 % S != 0:
            gtwo = host_gated([t0 - 1, t0 - 2])            # [2, H]
            bh[0] = gtwo[0] * w1 * SG                      # g1[t0-1]
            bh[1] = gtwo[1] * w0 * SG                      # g0[t0-2]
            bh[2] = gtwo[0] * w0 * SG                      # g0[t0-1]
        in_map["bhost"] = bh.astype(E4)

        in_map.update({
            "wkhi": wkhi, "wklo": wklo, "wvhi": wvhi, "wvlo": wvlo,
            "hid": np.ascontiguousarray(hid_bf[t0:t0 + TOK]),
            "w2p1bc": w2p1bc, "w1sbc": w1sbc, "w0sbc": w0sbc,
            "s12": s12, "eb": eb, "consts": consts,
        })
        in_maps.append(in_map)

    nc = _get_program()
    res = run_bass_kernel_spmd(nc, in_maps, list(range(N_CORES)))
    out = np.empty((B * S, H), np.float32)
    for r in range(N_CORES):
        out[r * TOK:(r + 1) * TOK] = res.results[r]["out"].astype(np.float32)
    return out.reshape(B, S, H)


# revision 58
# speedup vs baseline: 1.5906x; 1.0013x over previous
"""EngramModule kernel for Trainium2 (8 NeuronCores, SPMD data-parallel).

v2 architecture (fp8 DoubleRow 3-term GEMMs + transpose-gather):

Per token t (feature dim H=2048):
  idx[t, h]   = hash of n-gram ending at t (8 heads; computed on host)
  memory[t]   = concat_h tables[h, idx[t, h]]
  key_raw     = memory @ Wk.T ; value_raw = memory @ Wv.T
  gate        = sigmoid(dot(hidden, key_raw)/(sqrt(H)*rms_k) - 4)
  g[t]        = gate * value_raw / rms_v           (value_norm folded in conv w)
  out[t]      = g[t]*(1+w2) + w1*g[t-1] + w0*g[t-2]

Device strategy per core (2048 tokens, 16 tiles of 128):
 - Tables are compacted on host per (core, head) to <=2048 unique rows and
   packed as fp8 e4m3 (hi, lo) byte-interleaved 512B rows. One dma_gather
   with transpose=True per (head, segment) delivers memory ALREADY in lhsT
   layout [k-dim on partitions, tokens on free], with (hi, lo) as the two
   bytes of each 16-bit transpose unit -> directly usable as DoubleRow
   operand slots.
 - GEMMs run as 3-term compensated fp8 DoubleRow matmuls (error ~0.1%):
     (Mhi 2-slab)x(Whi 2-slab) + (Mlo)x(Whi) + (Mhi)x(Wlo)
   at 0.25 PE-cycles per output column per 128-contraction.
 - Phase 1 (key): per tile accumulate k in PSUM, ACT-square sumsq + DVE dot
   with bf16 hidden, Newton rsqrt, one ACT Sigmoid -> per-token gate.
 - Phase 2 (value): v in PSUM, sumsq -> rsq_v, g = ACT copy(v * s) bf16;
   g1/g0 cast to fp8; conv = one fp8-DR matmul (shift-pair slots) + one
   fp8-DR boundary matmul per 512-chunk; out = ACT copy(conv psum) + g2.
 - Conv boundary rows for tile 0 are computed EXACTLY on host (2 tokens).
 - Output written bf16, upcast on host.
Only ACT funcs {Square, Sigmoid, Copy} are used -> single act table set,
zero LoadActFuncSet reloads.
"""

import sys

import numpy as np

try:
    import concourse.bass as bass  # noqa: F401
except ImportError:
    sys.path.insert(0, "/opt/trn_rl_repo")

import ml_dtypes

import concourse.bacc as bacc
import concourse.bass as bass
import concourse.tile as tile
from concourse import mybir
from concourse.bass_utils import run_bass_kernel_spmd

E4 = ml_dtypes.float8_e4m3fn
BFNP = ml_dtypes.bfloat16
F32 = mybir.dt.float32
BF16 = mybir.dt.bfloat16
FP8 = mybir.dt.float8e4
I16 = mybir.dt.int16
I32 = mybir.dt.int32
ALU = mybir.AluOpType
AF = mybir.ActivationFunctionType
DR = mybir.MatmulPerfMode.DoubleRow

P = 128
H = 2048
HEADS = 8
HEAD_DIM = 256
VOCAB = 65536
MODULUS = VOCAB - 1
EPS = 1e-6
GATE_BIAS = -4.0
N_CORES = 8
B, S = 4, 4096
TOK = (B * S) // N_CORES        # 2048 tokens per core
NT = TOK // P                   # 16 tiles
CROWS = 2048                    # compact table rows per (core, head)
# gathers batch 4 heads x 256 tokens = 1024 indices (fits the 1024-slot
# SWDGE ring) against the concatenated per-core table [8*2048, 512]
GTOK = 256                      # tokens per gather
NRANGE = TOK // GTOK            # 8 token ranges
NHG = 4                         # head groups of 2
SG = 128.0                      # fp8 scale for g1/g0 (conv operands)
FMAX = 64.0                     # fp8 operand absmax (PSUM partial < ~5.5e4)
NCH = H // 512                  # 4 col chunks of 512


# ---------------------------------------------------------------- host hashing
def _hash_ids_np(ids, mult, off, n):
    """Exact replica of the reference _hash_ids in numpy (wrapping int64)."""
    Bb, Ss = ids.shape
    nh = mult.shape[0]
    ids_u = ids.astype(np.uint64)
    mult_u = mult.astype(np.uint64)
    off_u = off.astype(np.uint64)
    mix = np.zeros((Bb, Ss, nh), dtype=np.uint64)
    for p in range(n):
        shift = n - 1 - p
        tok = np.zeros_like(ids_u)
        if shift > 0:
            tok[:, shift:] = ids_u[:, : Ss - shift]
        else:
            tok = ids_u
        mix ^= tok[:, :, None] * mult_u[None, None, :, p]
    h = (mix + off_u[None, None, :]).view(np.int64)
    hmod = np.remainder(h, MODULUS) + 1
    valid = (np.arange(Ss) >= n - 1)[None, :, None]
    return np.where(valid, hmod, 0)


def _global_indices(input_ids, hm2, ho2, hm3, ho3):
    """[B, S, 8] int32 row indices into the flattened [8*65536, 256] table."""
    h2 = _hash_ids_np(input_ids, hm2, ho2, 2)
    h3 = _hash_ids_np(input_ids, hm3, ho3, 3)
    hid = np.concatenate([h2, h3], axis=-1)          # [B, S, 8]
    gidx = hid + (np.arange(HEADS, dtype=np.int64) * VOCAB)[None, None, :]
    return gidx.astype(np.int32)


def _wrap_idx(inv, n_tok):
    """int16 idx tile [128, n_tok//16]: slot i lives at [i%16, i//16]."""
    t = np.zeros((16, n_tok // 16), np.int16)
    t[np.arange(n_tok) % 16, np.arange(n_tok) // 16] = inv.astype(np.int16)
    return np.ascontiguousarray(np.tile(t, (8, 1)))


# ---------------------------------------------------------------- device program
def build_program():
    nc = bacc.Bacc(None, target_bir_lowering=False)

    t_ctab = nc.dram_tensor("ctab", [HEADS * CROWS, 512], FP8,
                            kind="ExternalInput")
    # idx tiles batched into two tensors: tiny range-0 block loads first so
    # the first gathers start immediately. 64 int16 words per gather.
    IDXW0 = (HEADS // NHG * GTOK) // 16 * NHG
    IDXWR = (HEADS // NHG * GTOK) // 16 * NHG * (NRANGE - 1)
    t_idx0 = nc.dram_tensor("idx0", [P, IDXW0], I16, kind="ExternalInput")
    t_idxr = nc.dram_tensor("idxr", [P, IDXWR], I16, kind="ExternalInput")
    t_wkhi = nc.dram_tensor("wkhi", [P, HEADS, 2, H], FP8, kind="ExternalInput")
    t_wklo = nc.dram_tensor("wklo", [P, HEADS, 2, H], FP8, kind="ExternalInput")
    t_wvhi = nc.dram_tensor("wvhi", [P, HEADS, 2, H], FP8, kind="ExternalInput")
    t_wvlo = nc.dram_tensor("wvlo", [P, HEADS, 2, H], FP8, kind="ExternalInput")
    t_hid = nc.dram_tensor("hid", [TOK, H], BF16, kind="ExternalInput")
    t_w2p1 = nc.dram_tensor("w2p1bc", [P, H], BF16, kind="ExternalInput")
    t_w1s = nc.dram_tensor("w1sbc", [P, H], BF16, kind="ExternalInput")
    t_w0s = nc.dram_tensor("w0sbc", [P, H], BF16, kind="ExternalInput")
    t_s12 = nc.dram_tensor("s12", [P, 2, P], FP8, kind="ExternalInput")
    t_eb = nc.dram_tensor("eb", [P, 2, P], FP8, kind="ExternalInput")
    t_bh = nc.dram_tensor("bhost", [4, H], FP8, kind="ExternalInput")
    t_out = nc.dram_tensor("out", [TOK, H], BF16, kind="ExternalOutput")

    # scalar constants (host-computed, folded scales)
    t_consts = nc.dram_tensor("consts", [P, 4], F32, kind="ExternalInput")
    # consts columns: 0 = eps_k' = (sM*sWk)^2 * H * EPS
    #                 1 = eps_v' = (sM*sWv)^2 * H * EPS
    #                 2 = sqrt(H)
    #                 3 = GATE_BIAS

    with tile.TileContext(nc) as tc:
        with (
            tc.tile_pool(name="pconst", bufs=1) as pc,
            tc.tile_pool(name="pM", bufs=1) as pM,
            tc.tile_pool(name="pWvpre", bufs=1) as pWvpre,
            tc.tile_pool(name="pstat", bufs=4) as pst,
            tc.tile_pool(name="pscr", bufs=2) as pscr,
            tc.tile_pool(name="pbig", bufs=3, space="PSUM") as pbig,
        ):
            # ---- small consts
            idx0_sb = pc.tile([P, IDXW0], I16)
            nc.sync.dma_start(out=idx0_sb[:], in_=t_idx0[:])
            consts = pc.tile([P, 4], F32)
            nc.sync.dma_start(out=consts[:], in_=t_consts[:])
            idxr_sb = pc.tile([P, IDXWR], I16)
            nc.sync.dma_start(out=idxr_sb[:], in_=t_idxr[:])
            s12_sb = pc.tile([P, 2, P], FP8)
            eb_sb = pc.tile([P, 2, P], FP8)
            sg_all = pc.tile([P, NT], F32)      # per-tile gate scalars

            # ---- gathers: (token-range, head-group) rectangles of 1024 idx
            HPG = HEADS // NHG  # heads per gather
            NIG = HPG * GTOK    # idxs per gather
            mseg = [[None] * NHG for _ in range(NRANGE)]

            def emit_gathers(ranges):
                for r in ranges:
                    for hg in range(NHG):
                        g = r * NHG + hg
                        W = NIG // 16
                        if r == 0:
                            iap = idx0_sb[:, hg * W:(hg + 1) * W]
                        else:
                            w0 = (g - NHG) * W
                            iap = idxr_sb[:, w0:w0 + W]
                        m = pM.tile([P, 4 * NIG], FP8, name=f"m{r}_{hg}")
                        nc.gpsimd.dma_gather(
                            out_ap=m[:].rearrange("p (f t) -> p f t", f=4),
                            in_ap=t_ctab[:],
                            idxs_ap=iap,
                            num_idxs=NIG, num_idxs_reg=NIG,
                            elem_size=512, transpose=True)
                        mseg[r][hg] = m

            emit_gathers(range(2))

            def m_slabs(h, i):
                """(lhsT_hi, lhsT_lo) [p, c(2), t(128)] for tile i, head h."""
                m = mseg[i // 2][h // (HEADS // NHG)]
                t0 = (h % (HEADS // NHG)) * GTOK + (i % 2) * P
                ctj = m[:].rearrange("p (c t j) -> p c t j", c=2, t=NIG, j=2)
                return (ctj[:, :, t0:t0 + P, 0], ctj[:, :, t0:t0 + P, 1])

            def gemm_terms(i, whi_sb, wlo_sb, ph, terms, start, stop,
                           chunks=tuple(range(NCH))):
                """Emit a subset of the 3-term fp8 DR GEMM for tile i into
                psum halves ph[0|1]. Terms: 0 = Mhi@Whi, 1 = Mlo@Whi,
                2 = Mhi@Wlo. start/stop apply at the first/last emitted
                matmul of each psum chunk group."""
                for tx, term in enumerate(terms):
                    for h in range(HEADS):
                        hi, lo = m_slabs(h, i)
                        mop = hi if term != 1 else lo
                        wsb = whi_sb if term != 2 else wlo_sb
                        for ch in chunks:
                            pt = ph[ch // 2]
                            cs = slice((ch % 2) * 512, (ch % 2) * 512 + 512)
                            nc.tensor.matmul(
                                pt[:, cs], lhsT=mop,
                                rhs=wsb[:, h, :, ch * 512:(ch + 1) * 512],
                                start=(start and tx == 0 and h == 0),
                                stop=(stop and tx == len(terms) - 1
                                      and h == HEADS - 1),
                                perf_mode=DR)

            def newton_rsqrt(u):
                """In-place u <- 1/sqrt(u) via bit-seed + 2 Newton iters."""
                y = pst.tile([P, 1], F32, tag="ny")
                yi = y[:].bitcast(I32)
                nc.vector.tensor_scalar(out=yi, in0=u[:].bitcast(I32), scalar1=1,
                                        scalar2=None, op0=ALU.logical_shift_right)
                nc.vector.tensor_scalar(out=yi, in0=yi, scalar1=-1,
                                        scalar2=0x5F3759DF, op0=ALU.mult,
                                        op1=ALU.add)
                t2 = pst.tile([P, 1], F32, tag="nt")
                for _ in range(2):
                    nc.vector.tensor_mul(out=t2[:], in0=y[:], in1=y[:])
                    nc.vector.tensor_mul(out=t2[:], in0=t2[:], in1=u[:])
                    nc.vector.tensor_scalar(out=t2[:], in0=t2[:], scalar1=-0.5,
                                            scalar2=1.5, op0=ALU.mult, op1=ALU.add)
                    nc.vector.tensor_mul(out=y[:], in0=y[:], in1=t2[:])
                return y

            # ================= PHASE 1: key =================
            wvlo_sb = pWvpre.tile([P, HEADS, 2, H], FP8)

            def key_half_stats(ph, hid_sb, sq2, dt2, x):
                scr = pscr.tile([P, 1024], F32, tag="scr")
                nc.scalar.activation(out=scr[:], in_=ph[x][:],
                                     func=AF.Square,
                                     accum_out=sq2[:, x:x + 1])
                scr = pscr.tile([P, 1024], F32, tag="scr")
                nc.vector.scalar_tensor_tensor(
                    out=scr[:], in0=ph[x][:], scalar=1.0,
                    in1=hid_sb[:, x * 1024:(x + 1) * 1024],
                    op0=ALU.mult, op1=ALU.mult,
                    accum_out=dt2[:, x:x + 1])

            def key_epi(i, ph, hid_sb, sq2=None, dt2=None):
                if sq2 is None:
                    sq2 = pst.tile([P, 2], F32, tag="sq2")
                    dt2 = pst.tile([P, 2], F32, tag="dt2")
                    for x in range(2):
                        key_half_stats(ph, hid_sb, sq2, dt2, x)
                u = pst.tile([P, 1], F32, tag="u")
                nc.vector.tensor_reduce(out=u[:], in_=sq2[:],
                                        axis=mybir.AxisListType.X, op=ALU.add)
                # u = sumsq + eps_k'  (rsq' = 1/sqrt(u) folds the /sqrt(H))
                nc.vector.tensor_scalar(out=u[:], in0=u[:],
                                        scalar1=consts[:, 0:1], scalar2=None,
                                        op0=ALU.add)
                rsq = newton_rsqrt(u)
                dot = pst.tile([P, 1], F32, tag="dot")
                nc.vector.tensor_reduce(out=dot[:], in_=dt2[:],
                                        axis=mybir.AxisListType.X, op=ALU.add)
                # gate = sigmoid(dot * rsq' - 4)
                nc.scalar.activation(out=sg_all[:, i:i + 1], in_=dot[:],
                                     func=AF.Sigmoid, scale=rsq[:],
                                     bias=consts[:, 3:4])

            with (
                tc.tile_pool(name="pWk", bufs=1) as pWk,
                tc.tile_pool(name="phid", bufs=3) as phid,
                tc.tile_pool(name="pkx", bufs=1, space="PSUM") as pkx,
            ):
                wkhi_sb = pWk.tile([P, HEADS, 2, H], FP8)
                wklo_sb = pWk.tile([P, HEADS, 2, H], FP8)
                for h in range(HEADS):
                    nc.sync.dma_start(out=wkhi_sb[:, h], in_=t_wkhi[:, h])
                # Wk_lo on the Pool queue, sequenced after the range-0/1
                # gathers so the early tiles' gathers win the DMA FIFO
                for h in range(HEADS):
                    nc.gpsimd.dma_start(out=wklo_sb[:, h], in_=t_wklo[:, h])
                emit_gathers(range(2, NRANGE))

                # tiles 0 and 1 are software-pipelined term-wise: their Whi
                # terms run while Wk_lo is still streaming in
                hid01 = []
                ph01 = []
                for i in range(2):
                    hid_sb = phid.tile([P, H], BF16, tag="hid")
                    nc.sync.dma_start(out=hid_sb[:],
                                      in_=t_hid[i * P:(i + 1) * P, :])
                    hid01.append(hid_sb)
                ph01.append([pbig.tile([P, 1024], F32, tag="ps", name="k0_0"),
                             pbig.tile([P, 1024], F32, tag="ps", name="k0_1")])
                ph01.append([pbig.tile([P, 1024], F32, tag="ps", name="k1_0"),
                             pkx.tile([P, 1024], F32, name="k1_1")])
                gemm_terms(0, wkhi_sb, wklo_sb, ph01[0], (0, 1), True, False)
                gemm_terms(1, wkhi_sb, wklo_sb, ph01[1], (0, 1), True, False)
                gemm_terms(0, wkhi_sb, wklo_sb, ph01[0], (2,), False, True)
                key_epi(0, ph01[0], hid01[0])
                gemm_terms(1, wkhi_sb, wklo_sb, ph01[1], (2,), False, True)
                key_epi(1, ph01[1], hid01[1])

                for i in range(2, NT):
                    if i == 8:
                        # mid-key prefetch of Wv_lo (Pool queue; DMA engines
                        # are free of startup traffic by now)
                        for h in range(HEADS):
                            nc.gpsimd.dma_start(out=wvlo_sb[:, h],
                                                in_=t_wvlo[:, h])
                    hid_sb = phid.tile([P, H], BF16, tag="hid")
                    nc.sync.dma_start(out=hid_sb[:],
                                      in_=t_hid[i * P:(i + 1) * P, :])
                    ph = [pbig.tile([P, 1024], F32, tag="ps", name=f"k{i}_{x}")
                          for x in range(2)]
                    sq2 = pst.tile([P, 2], F32, tag="sq2")
                    dt2 = pst.tile([P, 2], F32, tag="dt2")
                    # half-major: half 0's stats overlap half 1's GEMM
                    for x in range(2):
                        gemm_terms(i, wkhi_sb, wklo_sb, ph, (0, 1, 2),
                                   True, True, chunks=(2 * x, 2 * x + 1))
                        key_half_stats(ph, hid_sb, sq2, dt2, x)
                    key_epi(i, ph, hid_sb, sq2, dt2)

            # ================= PHASE 2: value =================
            with (
                tc.tile_pool(name="pWvhi", bufs=1) as pWvhi,
                tc.tile_pool(name="pg", bufs=2) as pg,
                tc.tile_pool(name="pout", bufs=2) as pout,
                tc.tile_pool(name="pb", bufs=1) as pb,
                tc.tile_pool(name="pconv", bufs=2, space="PSUM") as pconv,
            ):
                # Wv_hi streamed on SP first; the small constant tiles after
                # it so they don't steal DMA-engine slots from the stream
                wvhi_sb = pWvhi.tile([P, HEADS, 2, H], FP8)
                for h in range(HEADS):
                    nc.sync.dma_start(out=wvhi_sb[:, h], in_=t_wvhi[:, h])
                w2p1_sb = pWvhi.tile([P, H], BF16)
                w1s_sb = pWvhi.tile([P, H], BF16)
                w0s_sb = pWvhi.tile([P, H], BF16)
                nc.sync.dma_start(out=w2p1_sb[:], in_=t_w2p1[:])
                nc.sync.dma_start(out=w1s_sb[:], in_=t_w1s[:])
                nc.sync.dma_start(out=w0s_sb[:], in_=t_w0s[:])
                nc.sync.dma_start(out=s12_sb[:], in_=t_s12[:])
                nc.sync.dma_start(out=eb_sb[:], in_=t_eb[:])

                # boundary ping-pong tiles (fixed, fully memset once so reads
                # of untouched rows are well-defined); tile 0's rows from host
                b_tiles = [pb.tile([P, 2, H], FP8, name=f"b{x}")
                           for x in range(2)]
                nc.vector.memset(b_tiles[0][:], 0.0)
                nc.vector.memset(b_tiles[1][:], 0.0)
                nc.sync.dma_start(out=b_tiles[0][0:4, 0, :], in_=t_bh[:])

                for i in range(NT):
                    b_prev = b_tiles[i % 2]
                    ph = [pbig.tile([P, 1024], F32, tag="ps", name=f"v{i}_{x}")
                          for x in range(2)]
                    sq2 = pst.tile([P, 2], F32, tag="sq2")
                    # half-major: half 0's sumsq overlaps half 1's GEMM
                    for x in range(2):
                        gemm_terms(i, wvhi_sb, wvlo_sb, ph, (2, 0, 1),
                                   True, True, chunks=(2 * x, 2 * x + 1))
                        scr = pscr.tile([P, 1024], F32, tag="scr")
                        nc.scalar.activation(out=scr[:], in_=ph[x][:],
                                             func=AF.Square,
                                             accum_out=sq2[:, x:x + 1])
                    u = pst.tile([P, 1], F32, tag="u")
                    nc.vector.tensor_reduce(out=u[:], in_=sq2[:],
                                            axis=mybir.AxisListType.X,
                                            op=ALU.add)
                    nc.vector.tensor_scalar(out=u[:], in0=u[:],
                                            scalar1=consts[:, 1:2], scalar2=None,
                                            op0=ALU.add)
                    rsq = newton_rsqrt(u)
                    # s_final = rsq * sqrt(H) * gate
                    sfin = pst.tile([P, 1], F32, tag="sfin")
                    nc.vector.scalar_tensor_tensor(
                        out=sfin[:], in0=rsq[:], scalar=consts[:, 2:3],
                        in1=sg_all[:, i:i + 1], op0=ALU.mult, op1=ALU.mult)
                    # per-half epilogue pipeline: g -> g2/g10 -> conv -> copy
                    # -> add -> out DMA, so half 0 drains while half 1 is
                    # still in its GEMM/stat stages (shortens the end tail)
                    g = pg.tile([P, H], BF16, tag="g")
                    g2 = pg.tile([P, H], BF16, tag="g2")
                    g10 = pg.tile([P, 2, H], FP8, tag="g10")
                    out_sb = pout.tile([P, H], BF16, tag="out")
                    for x in range(2):
                        hs = slice(x * 1024, (x + 1) * 1024)
                        nc.scalar.activation(out=g[:, hs], in_=ph[x][:],
                                             func=AF.Copy, scale=sfin[:])
                        nc.vector.tensor_mul(out=g10[:, 0, hs], in0=g[:, hs],
                                             in1=w1s_sb[:, hs])
                        nc.vector.tensor_mul(out=g10[:, 1, hs], in0=g[:, hs],
                                             in1=w0s_sb[:, hs])
                        nc.vector.tensor_mul(out=g2[:, hs], in0=g[:, hs],
                                             in1=w2p1_sb[:, hs])
                        for ch in (2 * x, 2 * x + 1):
                            cs = slice(ch * 512, (ch + 1) * 512)
                            pcv = pconv.tile([P, 512], F32, tag="pcv")
                            nc.tensor.matmul(pcv[:], lhsT=s12_sb[:],
                                             rhs=g10[:, :, cs],
                                             start=True, stop=False,
                                             perf_mode=DR)
                            nc.tensor.matmul(pcv[:], lhsT=eb_sb[:],
                                             rhs=b_prev[:, :, cs],
                                             start=False, stop=True,
                                             perf_mode=DR)
                            nc.scalar.activation(out=out_sb[:, cs], in_=pcv[:],
                                                 func=AF.Copy)
                        nc.vector.tensor_add(out=out_sb[:, hs],
                                             in0=out_sb[:, hs],
                                             in1=g2[:, hs])
                        nc.sync.dma_start(out=t_out[i * P:(i + 1) * P, hs],
                                          in_=out_sb[:, hs])

                    # boundary rows for next tile from g10 tails
                    if i < NT - 1:
                        b_next = b_tiles[(i + 1) % 2]
                        nc.sync.dma_start(out=b_next[0:1, 0, :],
                                          in_=g10[127:128, 0, :])
                        nc.sync.dma_start(out=b_next[1:3, 0, :],
                                          in_=g10[126:128, 1, :])

    nc.compile()
    return nc


# ---------------------------------------------------------------- host wrapper
_PROGRAM = None


def _get_program():
    global _PROGRAM
    if _PROGRAM is None:
        _PROGRAM = build_program()
    return _PROGRAM


def kernel(hidden_states, input_ids, tables, Wk, Wv, key_norm_w, value_norm_w,
           conv_w, hm2, ho2, hm3, ho3):
    hidden_states = np.asarray(hidden_states, dtype=np.float32)
    input_ids = np.asarray(input_ids, dtype=np.int64)
    tables = np.asarray(tables, dtype=np.float32)
    Wk = np.asarray(Wk, dtype=np.float32)
    Wv = np.asarray(Wv, dtype=np.float32)
    key_norm_w = np.asarray(key_norm_w, dtype=np.float32)
    value_norm_w = np.asarray(value_norm_w, dtype=np.float32)
    conv_w = np.asarray(conv_w, dtype=np.float32)

    gidx = _global_indices(input_ids, np.asarray(hm2), np.asarray(ho2),
                           np.asarray(hm3), np.asarray(ho3))   # [B,S,8] i32
    hid_local = (gidx % VOCAB).reshape(B * S, HEADS)           # per-head rows

    # fold key_norm into hidden; fp8 scales
    hid_flat = (hidden_states.reshape(B * S, H) * key_norm_w[None, :])
    hid_bf = hid_flat.astype(BFNP)

    sM = FMAX / max(np.abs(tables).max(), 1e-30)
    sWk = FMAX / max(np.abs(Wk).max(), 1e-30)
    sWv = FMAX / max(np.abs(Wv).max(), 1e-30)

    def split_fp8(x, s):
        hi = (x * s).astype(E4)
        lo = ((x * s) - hi.astype(np.float32)).astype(E4)
        return hi, lo

    def w_layout(W, s):
        # [p, head, c, n] with W.T[k, n] = W[n, k]; k = h*256 + c*128 + p
        hi, lo = split_fp8(np.ascontiguousarray(W.T), s)   # [k, n]
        def lay(a):
            return np.ascontiguousarray(
                a.reshape(HEADS, 2, P, H).transpose(2, 0, 1, 3))
        return lay(hi), lay(lo)

    wkhi, wklo = w_layout(Wk, sWk)
    wvhi, wvlo = w_layout(Wv, sWv)

    # conv weight foldings (value_norm + fp8 g-scale)
    w0 = conv_w[:, 0] * value_norm_w
    w1 = conv_w[:, 1] * value_norm_w
    w2p1 = (1.0 + conv_w[:, 2]) * value_norm_w
    w2p1bc = np.ascontiguousarray(np.broadcast_to(w2p1, (P, H))).astype(BFNP)
    w1sbc = np.ascontiguousarray(np.broadcast_to(w1 * SG, (P, H))).astype(BFNP)
    w0sbc = np.ascontiguousarray(np.broadcast_to(w0 * SG, (P, H))).astype(BFNP)

    # shift-pair (s1, s2) and boundary lhsT matrices, scaled by 1/SG
    inv = np.float32(1.0 / SG)
    s12 = np.zeros((P, 2, P), E4)
    s12[:, 0, :] = (np.eye(P, k=1, dtype=np.float32) * inv).astype(E4)
    s12[:, 1, :] = (np.eye(P, k=2, dtype=np.float32) * inv).astype(E4)
    eb = np.zeros((P, 2, P), E4)
    # b_pad rows: 0 -> out0 (g1[127]), 1 -> out0 (g0[126]), 2 -> out1 (g0[127])
    eb[0, 0, 0] = E4(inv)
    eb[1, 0, 0] = E4(inv)
    eb[2, 0, 1] = E4(inv)

    consts = np.zeros((P, 4), np.float32)
    consts[:, 0] = (sM * sWk) ** 2 * H * EPS
    consts[:, 1] = (sM * sWv) ** 2 * H * EPS
    consts[:, 2] = np.sqrt(np.float32(H))
    consts[:, 3] = GATE_BIAS

    # exact host reference for the 2 boundary tokens of each core
    def host_gated(trange):
        """gated[t] rows (f64->f32) for global token indices trange."""
        out = np.zeros((len(trange), H), np.float32)
        tabs = tables.astype(np.float64)
        for j, t in enumerate(trange):
            rows = [tabs[h, hid_local[t, h]] for h in range(HEADS)]
            mem = np.concatenate(rows)                     # [2048]
            kr = mem @ Wk.T.astype(np.float64)
            vr = mem @ Wv.T.astype(np.float64)
            rk = 1.0 / np.sqrt(np.mean(kr ** 2) + EPS)
            rv = 1.0 / np.sqrt(np.mean(vr ** 2) + EPS)
            z = float(hid_flat[t].astype(np.float64) @ (kr * rk)) / np.sqrt(H) \
                + GATE_BIAS
            gate = 1.0 / (1.0 + np.exp(-z))
            out[j] = (gate * (vr * rv) * value_norm_w).astype(np.float32)
        return out

    in_maps = []
    for r in range(N_CORES):
        t0 = r * TOK
        idx_core = hid_local[t0:t0 + TOK]                  # [2048, 8]
        in_map = {}
        packed = np.zeros((HEADS * CROWS, 512), np.uint8)
        gidx16 = np.empty((TOK, HEADS), np.int16)          # h*2048 + inv
        for h in range(HEADS):
            uniq, invmap = np.unique(idx_core[:, h], return_inverse=True)
            rows = tables[h, uniq]                         # [n_u, 256]
            hi, lo = split_fp8(rows, sM)
            blk = packed[h * CROWS:h * CROWS + len(uniq)]
            blk[:, 0::2] = hi.view(np.uint8)
            blk[:, 1::2] = lo.view(np.uint8)
            gidx16[:, h] = (h * CROWS + invmap).astype(np.int16)
        in_map["ctab"] = packed.view(E4)
        # gather idx rectangles: (token-range r, head-group hg): the 1024
        # idxs are head-major (4 heads x 256 tokens) to match m_slabs
        idx_cols = []
        for r_ in range(NRANGE):
            for hg in range(NHG):
                hpg = HEADS // NHG
                rect = gidx16[r_ * GTOK:(r_ + 1) * GTOK,
                              hg * hpg:(hg + 1) * hpg]
                seq = np.ascontiguousarray(rect.T).reshape(-1)   # head-major
                idx_cols.append(_wrap_idx(seq, hpg * GTOK))
        in_map["idx0"] = np.ascontiguousarray(
            np.concatenate(idx_cols[:NHG], axis=1))
        in_map["idxr"] = np.ascontiguousarray(
            np.concatenate(idx_cols[NHG:], axis=1))

        # boundary rows for tile 0
        bh = np.zeros((4, H), np.float32)
        if t0 % S != 0:
            gtwo = host_gated([t0 - 1, t0 - 2])            # [2, H]
            bh[0] = gtwo[0] * w1 * SG                      # g1[t0-1]
            bh[1] = gtwo[1] * w0 * SG                      # g0[t0-2]
            bh[2] = gtwo[0] * w0 * SG                      # g0[t0-1]
        in_map["bhost"] = bh.astype(E4)

        in_map.update({
            "wkhi": wkhi, "wklo": wklo, "wvhi": wvhi, "wvlo": wvlo,
            "hid": np.ascontiguousarray(hid_bf[t0:t0 + TOK]),
            "w2p1bc": w2p1bc, "w1sbc": w1sbc, "w0sbc": w0sbc,
            "s12": s12, "eb": eb, "consts": consts,
        })
        in_maps.append(in_map)

    nc = _get_program()
    res = run_bass_kernel_spmd(nc, in_maps, list(range(N_CORES)))
    out = np.empty((B * S, H), np.float32)
    for r in range(N_CORES):
        out[r * TOK:(r + 1) * TOK] = res.results[r]["out"].astype(np.float32)
    return out.reshape(B, S, H)
